# revision 1
# baseline (speedup 1.0000x reference)
"""GraphSAGE 2-layer fraud detector on 8 trn2 NeuronCores.

Strategy (dst-partitioned, matmul scatter):
  - Nodes padded to 50176 = 8 cores x 49 blocks x 128. Core c owns dst rows
    [c*6272, (c+1)*6272). Edges sorted by dst on host; each core gets the
    edges targeting its rows, grouped per 128-node dst block, chunked into
    128-edge chunks.
  - Layer 1 per chunk: indirect-DMA gather x[src] rows (512B each), build
    one-hot P[e,d] = (ldst[e]==d) on DVE, PSUM-accumulate P.T @ msg ->
    agg[dst,feat]. Mean via per-partition recip scale, then
    h = relu(agg@W1l.T + x@W1r.T + b1) computed feature-major (hT) via a PE
    transpose of agg.
  - z = h@W2l.T and o = h@W2r.T + b2 computed per block (mean-aggregation
    commutes with the linear map, so layer 2 aggregates the 2-wide z instead
    of the 256-wide h). z is AllGathered across cores (50KB/core); layer 2
    reuses the same chunk tables to gather z[src] rows and the same one-hot
    scatter into agg2[dst,2]. out = recip*agg2 + o.
"""

import time

import numpy as np

import concourse.bass as bass
import concourse.mybir as mybir
import concourse.tile as tile
from concourse import bacc
from concourse.bass_utils import run_bass_kernel_spmd

N = 50000
E = 800000
IN_C = 128
HID = 256
OUT_C = 2
NCORES = 8
P = 128
NB = 49                 # dst blocks per core
ROWS = NB * P           # 6272 rows per core
NP = NCORES * ROWS      # 50176 padded nodes
ZROWS = NCORES * P      # 1024 rows of the allgathered z tensor [1024, 2*NB]

f32 = mybir.dt.float32
i32 = mybir.dt.int32

DEBUG_TAPS = False


def _host_prep(x, edge_index, W1l, b1, W1r, W2l, b2, W2r):
    src = edge_index[0].astype(np.int64)
    dst = edge_index[1].astype(np.int64)
    cnt = np.bincount(dst, minlength=NP)
    recip = (1.0 / np.maximum(cnt, 1)).astype(np.float32)

    order = np.argsort(dst, kind="stable")
    s_src = src[order]
    s_dst = dst[order]

    block_starts = np.searchsorted(s_dst, np.arange(0, NP + P, P))
    cnt_blk = (block_starts[1:] - block_starts[:-1]).reshape(NCORES, NB)
    nb = np.maximum(1, -(-cnt_blk // P)).max(axis=0)  # [NB] chunks per block pos
    C1 = int(nb.sum())

    src_arr = np.full((NCORES, P, C1), N, dtype=np.int32)
    ldst_arr = np.full((NCORES, P, C1), 255, dtype=np.float32)
    col = 0
    for b in range(NB):
        w = int(nb[b])
        for c in range(NCORES):
            bb = c * NB + b
            s, e = int(block_starts[bb]), int(block_starts[bb + 1])
            k = e - s
            ts = np.full(w * P, N, np.int32)
            tl = np.full(w * P, 255, np.float32)
            ts[:k] = s_src[s:e]
            tl[:k] = s_dst[s:e] - bb * P
            src_arr[c, :, col:col + w] = ts.reshape(w, P).T
            ldst_arr[c, :, col:col + w] = tl.reshape(w, P).T
        col += w

    # layer-2 gathers the same edges from z_full, whose row layout is
    # [core, p, b]: node n lives at flat row (n//6272)*6272 + (n%128)*49
    # + ((n%6272)//128). Pad entries point at node N; their one-hot column
    # is zero so the gathered value never contributes.
    sa = src_arr.astype(np.int64)
    qsrc_arr = ((sa // ROWS) * ROWS + (sa % P) * NB
                + (sa % ROWS) // P).astype(np.int32)

    x_pad = np.zeros((NP + 1, IN_C), np.float32)
    x_pad[:N] = x
    W1lT = np.ascontiguousarray(W1l.T.astype(np.float32))   # [128, 256]
    W1rT = np.ascontiguousarray(W1r.T.astype(np.float32))
    Wzo = np.zeros((P, 8), np.float32)
    for j in range(2):
        Wzo[:, 4 * j:4 * j + 2] = W2l.T[j * P:(j + 1) * P, :]
        Wzo[:, 4 * j + 2:4 * j + 4] = W2r.T[j * P:(j + 1) * P, :]
    b1p = np.ascontiguousarray(np.asarray(b1).reshape(2, P).T.astype(np.float32))
    b2b = np.tile(np.asarray(b2).reshape(1, 2), (P, 1)).astype(np.float32)
    recip_c = recip.reshape(NCORES, NB, P).transpose(0, 2, 1).copy()  # [c,P,NB]
    iota = np.tile(np.arange(P, dtype=np.float32)[None, :], (P, 1))
    ident = np.eye(P, dtype=np.float32)

    in_maps = []
    for c in range(NCORES):
        xT_own = np.ascontiguousarray(
            x_pad[c * ROWS:(c + 1) * ROWS, :].T
        )  # [128, 6272]
        in_maps.append({
            "x_pad": x_pad,
            "src": np.ascontiguousarray(src_arr[c]),
            "ldst": np.ascontiguousarray(ldst_arr[c]),
            "qsrc": np.ascontiguousarray(qsrc_arr[c]),
            "xT_own": xT_own,
            "W1lT": W1lT,
            "W1rT": W1rT,
            "Wzo": Wzo,
            "b1p": b1p,
            "b2b": b2b,
            "recip": np.ascontiguousarray(recip_c[c]),
            "iota": iota,
            "ident": ident,
        })
    return in_maps, [int(v) for v in nb]


def _build(nb):
    C1 = sum(nb)
    nc = bacc.Bacc(None, target_bir_lowering=False, debug=False)

    x_pad_d = nc.dram_tensor("x_pad", [NP + 1, IN_C], f32, kind="ExternalInput")
    src_d = nc.dram_tensor("src", [P, C1], i32, kind="ExternalInput")
    ldst_d = nc.dram_tensor("ldst", [P, C1], f32, kind="ExternalInput")
    qsrc_d = nc.dram_tensor("qsrc", [P, C1], i32, kind="ExternalInput")
    xT_d = nc.dram_tensor("xT_own", [P, ROWS], f32, kind="ExternalInput")
    W1lT_d = nc.dram_tensor("W1lT", [P, HID], f32, kind="ExternalInput")
    W1rT_d = nc.dram_tensor("W1rT", [P, HID], f32, kind="ExternalInput")
    Wzo_d = nc.dram_tensor("Wzo", [P, 8], f32, kind="ExternalInput")
    b1p_d = nc.dram_tensor("b1p", [P, 2], f32, kind="ExternalInput")
    b2b_d = nc.dram_tensor("b2b", [P, 2], f32, kind="ExternalInput")
    recip_d = nc.dram_tensor("recip", [P, NB], f32, kind="ExternalInput")
    iota_d = nc.dram_tensor("iota", [P, P], f32, kind="ExternalInput")
    ident_d = nc.dram_tensor("ident", [P, P], f32, kind="ExternalInput")
    out_d = nc.dram_tensor("out", [P, 2 * NB], f32, kind="ExternalOutput")
    if DEBUG_TAPS:
        dbg_msg_d = nc.dram_tensor("dbg_msg", [P, nb[0] * P], f32,
                                   kind="ExternalOutput")
        dbg_aggm_d = nc.dram_tensor("dbg_aggm", [P, P], f32,
                                    kind="ExternalOutput")
        dbg_h0_d = nc.dram_tensor("dbg_h0", [P, P], f32, kind="ExternalOutput")
        dbg_z_d = nc.dram_tensor("dbg_z", [P, 2 * NB], f32,
                                 kind="ExternalOutput")

    with tile.TileContext(nc) as tc:
        with (
            tc.tile_pool(name="big", bufs=1) as big,
            tc.tile_pool(name="lp", bufs=4) as lp,
            tc.tile_pool(name="pp", bufs=2, space="PSUM") as pp,
            tc.tile_pool(name="dram", bufs=1, space="DRAM") as dp,
        ):
            def load(d, shape, dt, tag):
                t = big.tile(shape, dt, tag=tag)
                nc.sync.dma_start(out=t[:], in_=d[:, :])
                return t

            src_sb = load(src_d, [P, C1], i32, "src")
            ldst_sb = load(ldst_d, [P, C1], f32, "ldst")
            qsrc_sb = load(qsrc_d, [P, C1], i32, "qsrc")
            xT_sb = load(xT_d, [P, ROWS], f32, "xT")
            W1lT_sb = load(W1lT_d, [P, HID], f32, "w1l")
            W1rT_sb = load(W1rT_d, [P, HID], f32, "w1r")
            Wzo_sb = load(Wzo_d, [P, 8], f32, "wzo")
            b1_sb = load(b1p_d, [P, 2], f32, "b1")
            b2_sb = load(b2b_d, [P, 2], f32, "b2")
            recip_sb = load(recip_d, [P, NB], f32, "recip")
            iota_sb = load(iota_d, [P, P], f32, "iota")
            ident_sb = load(ident_d, [P, P], f32, "ident")

            hT = [
                big.tile([P, ROWS], f32, tag=f"hT{j}", name=f"hT{j}")
                for j in range(2)
            ]
            z_sb = big.tile([P, 2 * NB], f32, tag="z")
            o_sb = big.tile([P, 2 * NB], f32, tag="o")
            out_sb = big.tile([P, 2 * NB], f32, tag="outs")

            col = 0
            for b in range(NB):
                w = nb[b]
                pagg = pp.tile([P, P], f32, tag="agg")
                for k in range(w):
                    msg = lp.tile([P, P], f32, tag="msg")
                    nc.gpsimd.indirect_dma_start(
                        out=msg[:],
                        out_offset=None,
                        in_=x_pad_d[:, :],
                        in_offset=bass.IndirectOffsetOnAxis(
                            ap=src_sb[:, col + k:col + k + 1], axis=0
                        ),
                    )
                    if DEBUG_TAPS and b == 0:
                        nc.sync.dma_start(
                            out=dbg_msg_d[:, k * P:(k + 1) * P], in_=msg[:]
                        )
                    Pt = lp.tile([P, P], f32, tag="P")
                    nc.vector.tensor_scalar(
                        out=Pt[:], in0=iota_sb[:],
                        scalar1=ldst_sb[:, col + k:col + k + 1], scalar2=None,
                        op0=mybir.AluOpType.is_equal,
                    )
                    nc.tensor.matmul(
                        out=pagg[:], lhsT=Pt[:], rhs=msg[:],
                        start=(k == 0), stop=(k == w - 1),
                    )
                aggm = lp.tile([P, P], f32, tag="aggm")
                nc.vector.tensor_scalar(
                    out=aggm[:], in0=pagg[:], scalar1=recip_sb[:, b:b + 1],
                    scalar2=None, op0=mybir.AluOpType.mult,
                )
                if DEBUG_TAPS and b == 0:
                    nc.sync.dma_start(out=dbg_aggm_d[:, :], in_=aggm[:])
                ptr = pp.tile([P, P], f32, tag="tr")
                nc.tensor.transpose(out=ptr[:], in_=aggm[:], identity=ident_sb[:])
                aggmT = lp.tile([P, P], f32, tag="aggmT")
                nc.vector.tensor_copy(out=aggmT[:], in_=ptr[:])
                for j in range(2):
                    ph = pp.tile([P, P], f32, tag="h")
                    nc.tensor.matmul(
                        out=ph[:], lhsT=W1lT_sb[:, j * P:(j + 1) * P],
                        rhs=aggmT[:], start=True, stop=False,
                    )
                    nc.tensor.matmul(
                        out=ph[:], lhsT=W1rT_sb[:, j * P:(j + 1) * P],
                        rhs=xT_sb[:, b * P:(b + 1) * P], start=False, stop=True,
                    )
                    nc.scalar.activation(
                        out=hT[j][:, b * P:(b + 1) * P], in_=ph[:],
                        func=mybir.ActivationFunctionType.Relu,
                        bias=b1_sb[:, j:j + 1],
                    )
                if DEBUG_TAPS and b == 0:
                    nc.sync.dma_start(out=dbg_h0_d[:, :], in_=hT[0][:, 0:P])
                pzo = pp.tile([P, 4], f32, tag="zo")
                for j in range(2):
                    nc.tensor.matmul(
                        out=pzo[:], lhsT=hT[j][:, b * P:(b + 1) * P],
                        rhs=Wzo_sb[:, 4 * j:4 * j + 4],
                        start=(j == 0), stop=(j == 1),
                    )
                nc.vector.tensor_copy(out=z_sb[:, 2 * b:2 * b + 2], in_=pzo[:, 0:2])
                nc.vector.tensor_tensor(
                    out=o_sb[:, 2 * b:2 * b + 2], in0=pzo[:, 2:4], in1=b2_sb[:],
                    op=mybir.AluOpType.add,
                )
                col += w

            if DEBUG_TAPS:
                nc.sync.dma_start(out=dbg_z_d[:, :], in_=z_sb[:])

            # z -> DRAM, allgather
            z_own = dp.tile([P, 2 * NB], f32, tag="zown")
            nc.sync.dma_start(out=z_own[:], in_=z_sb[:])
            z_full = dp.tile([ZROWS, 2 * NB], f32, tag="zfull")
            nc.gpsimd.collective_compute(
                "AllGather",
                mybir.AluOpType.bypass,
                replica_groups=[list(range(NCORES))],
                ins=[z_own[:, :]],
                outs=[z_full[:, :]],
            )
            z_rows = z_full[:, :].rearrange("a (r f) -> (a r) f", f=2)

            col = 0
            for b in range(NB):
                w = nb[b]
                pa2 = pp.tile([P, 2], f32, tag="agg", name="pa2")
                for k in range(w):
                    zg = lp.tile([P, 2], f32, tag="zg")
                    nc.gpsimd.indirect_dma_start(
                        out=zg[:],
                        out_offset=None,
                        in_=z_rows,
                        in_offset=bass.IndirectOffsetOnAxis(
                            ap=qsrc_sb[:, col + k:col + k + 1], axis=0
                        ),
                    )
                    P2 = lp.tile([P, P], f32, tag="P2")
                    nc.vector.tensor_scalar(
                        out=P2[:], in0=iota_sb[:],
                        scalar1=ldst_sb[:, col + k:col + k + 1], scalar2=None,
                        op0=mybir.AluOpType.is_equal,
                    )
                    nc.tensor.matmul(
                        out=pa2[:], lhsT=P2[:], rhs=zg[:],
                        start=(k == 0), stop=(k == w - 1),
                    )
                red2 = lp.tile([P, 2], f32, tag="red2")
                nc.vector.tensor_scalar(
                    out=red2[:], in0=pa2[:], scalar1=recip_sb[:, b:b + 1],
                    scalar2=None, op0=mybir.AluOpType.mult,
                )
                nc.vector.tensor_tensor(
                    out=out_sb[:, 2 * b:2 * b + 2], in0=red2[:],
                    in1=o_sb[:, 2 * b:2 * b + 2], op=mybir.AluOpType.add,
                )
                col += w

            nc.sync.dma_start(out=out_d[:, :], in_=out_sb[:])
    nc.compile()
    return nc


def _run(inputs, repeat=1):
    in_maps, nb = _host_prep(**inputs)
    nc = _build(nb)
    best = None
    for _ in range(repeat):
        t0 = time.perf_counter()
        res = run_bass_kernel_spmd(
            nc, [dict(m) for m in in_maps], core_ids=list(range(NCORES))
        )
        dt = time.perf_counter() - t0
        best = dt if best is None else min(best, dt)
    outs = []
    for c in range(NCORES):
        a = res.results[c]["out"]  # [128, 98]
        outs.append(a.reshape(P, NB, 2).transpose(1, 0, 2).reshape(ROWS, 2))
    full = np.concatenate(outs, axis=0)[:N]
    return full.astype(np.float32), best


def kernel(**inputs):
    out, _ = _run(inputs, repeat=1)
    return out



# revision 2
# speedup vs baseline: 5.3205x; 5.3205x over previous
"""GraphSAGE 2-layer fraud detector on 8 trn2 NeuronCores.

Strategy (dst-partitioned, matmul scatter; wire-optimized):
  - The run is dominated by host->device transfer over the axon tunnel
    (~38MB/s per tensor-stream, streams transfer in parallel). So inputs are
    minimized and split into balanced streams: x is sharded by node rows
    (fp16, 8 split tensors per core => 16 parallel streams) and AllGathered
    on-device; edge tables ship as u16 src / u8 local-dst / fp16 per-edge
    reciprocal; weights ship fp16 packed.
  - Nodes padded to 50176 = 8 cores x 49 blocks x 128. Core c owns dst rows
    [c*6272, (c+1)*6272). Edges sorted by (dst block, src) on host; each core
    gets the edges targeting its rows, grouped per 128-node dst block,
    chunked into 128-edge chunks.
  - Layer 1 per chunk: indirect-DMA gather x_full[src] fp16 rows (256B),
    build scaled one-hot Pt[e,d] = (ldst[e]==d)*recip[dst[e]] on DVE (fp16),
    PSUM-accumulate msg.T @ Pt -> aggT[feat,dst] (feature-major, mean
    already applied; no transpose needed). h = relu(W1l@aggT + W1r@xT + b1)
    computed per 128-wide hid block; xT comes from DMA-transpose of the own
    x shard.
  - z = h@W2l.T and o = h@W2r.T + b2 per block (mean-aggregation commutes
    with the linear map, so layer 2 aggregates the 2-wide z instead of the
    256-wide h). z is written to DRAM in natural node order and AllGathered
    (50KB/core); layer 2 gathers z[src] with the SAME src table and scatters
    with the same scaled one-hot into agg2[dst,2]. out = agg2 + o.
"""

import os
import time

import numpy as np

import concourse.bass as bass
import concourse.mybir as mybir
import concourse.tile as tile
from concourse import bacc
from concourse.bass_utils import run_bass_kernel_spmd

N = 50000
E = 800000
IN_C = 128
HID = 256
OUT_C = 2
NCORES = 8
P = 128
NB = 49                 # dst blocks per core
ROWS = NB * P           # 6272 rows per core
NP = NCORES * ROWS      # 50176 padded nodes
NXS = 8                 # x shard splits (wire streams)
XSR = ROWS // NXS       # 784 rows per split

f32 = mybir.dt.float32
f16 = mybir.dt.float16
i32 = mybir.dt.int32
u16 = mybir.dt.uint16
u8 = mybir.dt.uint8


def _host_prep(x, edge_index, W1l, b1, W1r, W2l, b2, W2r):
    src = np.asarray(edge_index[0]).astype(np.int64)
    dst = np.asarray(edge_index[1]).astype(np.int64)
    cnt = np.bincount(dst, minlength=NP)
    recip = (1.0 / np.maximum(cnt, 1)).astype(np.float32)

    # sort by dst block, then src (DRAM locality for the gathers)
    order = np.lexsort((src, dst // P))
    s_src = src[order]
    s_dst = dst[order]
    s_blk = s_dst // P

    block_starts = np.searchsorted(s_blk, np.arange(0, NP // P + 1))
    cnt_blk = (block_starts[1:] - block_starts[:-1]).reshape(NCORES, NB)
    nb = np.maximum(1, -(-cnt_blk // P)).max(axis=0)  # [NB] chunks per block pos
    C1 = int(nb.sum())

    src_arr = np.full((NCORES, P, C1), N, dtype=np.uint16)
    ldst_arr = np.full((NCORES, P, C1), 255, dtype=np.uint8)
    rl_arr = np.zeros((NCORES, P, C1), dtype=np.float16)
    col = 0
    for b in range(NB):
        w = int(nb[b])
        for c in range(NCORES):
            bb = c * NB + b
            s, e = int(block_starts[bb]), int(block_starts[bb + 1])
            k = e - s
            ts = np.full(w * P, N, np.uint16)
            tl = np.full(w * P, 255, np.uint8)
            tr = np.zeros(w * P, np.float16)
            ts[:k] = s_src[s:e]
            tl[:k] = s_dst[s:e] - bb * P
            tr[:k] = recip[s_dst[s:e]]
            src_arr[c, :, col:col + w] = ts.reshape(w, P).T
            ldst_arr[c, :, col:col + w] = tl.reshape(w, P).T
            rl_arr[c, :, col:col + w] = tr.reshape(w, P).T
        col += w
    c1a = C1 // 2

    x_pad = np.zeros((NP, IN_C), np.float16)
    x_pad[:N] = np.asarray(x, dtype=np.float32).astype(np.float16)

    # packed fp16 weights: [128, 256+256+8+2+2]
    wpack = np.zeros((P, 2 * HID + 8 + 4), np.float16)
    wpack[:, 0:HID] = np.asarray(W1l).T.astype(np.float16)
    wpack[:, HID:2 * HID] = np.asarray(W1r).T.astype(np.float16)
    for j in range(2):
        wpack[:, 2 * HID + 4 * j:2 * HID + 4 * j + 2] = \
            np.asarray(W2l).T[j * P:(j + 1) * P, :].astype(np.float16)
        wpack[:, 2 * HID + 4 * j + 2:2 * HID + 4 * j + 4] = \
            np.asarray(W2r).T[j * P:(j + 1) * P, :].astype(np.float16)
    wpack[:, 2 * HID + 8:2 * HID + 10] = \
        np.asarray(b1).reshape(2, P).T.astype(np.float16)
    wpack[:, 2 * HID + 10:2 * HID + 12] = \
        np.tile(np.asarray(b2).reshape(1, 2), (P, 1)).astype(np.float16)

    in_maps = []
    for c in range(NCORES):
        xc = x_pad[c * ROWS:(c + 1) * ROWS]
        m = {
            "srcA": np.ascontiguousarray(src_arr[c, :, :c1a]),
            "srcB": np.ascontiguousarray(src_arr[c, :, c1a:]),
            "ldst": np.ascontiguousarray(ldst_arr[c]),
            "rlA": np.ascontiguousarray(rl_arr[c, :, :c1a]),
            "rlB": np.ascontiguousarray(rl_arr[c, :, c1a:]),
            "wpack": wpack,
        }
        for s in range(NXS):
            m[f"xs{s}"] = np.ascontiguousarray(xc[s * XSR:(s + 1) * XSR])
        in_maps.append(m)
    return in_maps, [int(v) for v in nb]


def _build(nb):
    C1 = sum(nb)
    c1a = C1 // 2
    WP = 2 * HID + 12
    nc = bacc.Bacc(None, target_bir_lowering=False, debug=False)

    xs_d = [
        nc.dram_tensor(f"xs{s}", [XSR, IN_C], f16, kind="ExternalInput")
        for s in range(NXS)
    ]
    srcA_d = nc.dram_tensor("srcA", [P, c1a], u16, kind="ExternalInput")
    srcB_d = nc.dram_tensor("srcB", [P, C1 - c1a], u16, kind="ExternalInput")
    ldst_d = nc.dram_tensor("ldst", [P, C1], u8, kind="ExternalInput")
    rlA_d = nc.dram_tensor("rlA", [P, c1a], f16, kind="ExternalInput")
    rlB_d = nc.dram_tensor("rlB", [P, C1 - c1a], f16, kind="ExternalInput")
    wpack_d = nc.dram_tensor("wpack", [P, WP], f16, kind="ExternalInput")
    out_d = nc.dram_tensor("out", [P, 2 * NB], f32, kind="ExternalOutput")

    with tile.TileContext(nc) as tc:
        with (
            tc.tile_pool(name="big", bufs=1) as big,
            tc.tile_pool(name="lp", bufs=4) as lp,
            tc.tile_pool(name="pp", bufs=2, space="PSUM") as pp,
            tc.tile_pool(name="ph", bufs=2, space="PSUM") as php,
            tc.tile_pool(name="dram", bufs=1, space="DRAM") as dp,
        ):
            # ---- input staging ----
            srcu = big.tile([P, C1], u16, tag="srcu")
            nc.sync.dma_start(out=srcu[:, :c1a], in_=srcA_d[:, :])
            nc.sync.dma_start(out=srcu[:, c1a:], in_=srcB_d[:, :])
            ldstu = big.tile([P, C1], u8, tag="ldstu")
            nc.sync.dma_start(out=ldstu[:], in_=ldst_d[:, :])
            rlh = big.tile([P, C1], f16, tag="rlh")
            nc.sync.dma_start(out=rlh[:, :c1a], in_=rlA_d[:, :])
            nc.sync.dma_start(out=rlh[:, c1a:], in_=rlB_d[:, :])
            wp_sb = big.tile([P, WP], f16, tag="wp")
            nc.sync.dma_start(out=wp_sb[:], in_=wpack_d[:, :])

            # own x shard -> contiguous DRAM (for AllGather) + SBUF xT (fp16)
            x_own = dp.tile([ROWS, IN_C], f16, tag="xown")
            x_full = dp.tile([NP, IN_C], f16, tag="xfull")
            xT = big.tile([P, ROWS], f16, tag="xT")
            for s in range(NXS):
                nc.sync.dma_start(
                    out=x_own[s * XSR:(s + 1) * XSR, :], in_=xs_d[s][:, :]
                )
                nc.scalar.dma_start(
                    out=xT[:, s * XSR:(s + 1) * XSR], in_=xs_d[s][:, :],
                    transpose=True,
                )
            nc.gpsimd.collective_compute(
                "AllGather",
                mybir.AluOpType.bypass,
                replica_groups=[list(range(NCORES))],
                ins=[x_own[:, :]],
                outs=[x_full[:, :]],
            )

            # ---- table conversions ----
            srci = big.tile([P, C1], i32, tag="srci")
            nc.vector.tensor_copy(out=srci[:], in_=srcu[:])
            ldstf = big.tile([P, C1], f32, tag="ldstf")
            nc.vector.tensor_copy(out=ldstf[:], in_=ldstu[:])
            rlf = big.tile([P, C1], f32, tag="rlf")
            nc.vector.tensor_copy(out=rlf[:], in_=rlh[:])
            b1f = big.tile([P, 2], f32, tag="b1f")
            nc.vector.tensor_copy(out=b1f[:], in_=wp_sb[:, 2 * HID + 8:2 * HID + 10])
            b2f = big.tile([P, 2], f32, tag="b2f")
            nc.vector.tensor_copy(out=b2f[:], in_=wp_sb[:, 2 * HID + 10:2 * HID + 12])

            iota_i = big.tile([P, P], i32, tag="iotai")
            nc.gpsimd.iota(out=iota_i[:], pattern=[[1, P]], base=0,
                           channel_multiplier=0)
            iota_f = big.tile([P, P], f32, tag="iotaf")
            nc.vector.tensor_copy(out=iota_f[:], in_=iota_i[:])

            hT = [
                big.tile([P, ROWS], f16, tag=f"hT{j}", name=f"hT{j}")
                for j in range(2)
            ]
            z_sb = big.tile([P, 2 * NB], f32, tag="z")
            o_sb = big.tile([P, 2 * NB], f32, tag="o")
            out_sb = big.tile([P, 2 * NB], f32, tag="outs")
            z_own = dp.tile([ROWS, 2], f32, tag="zown")
            z_full = dp.tile([NP, 2], f32, tag="zfull")

            # ---- layer 1 ----
            col = 0
            for b in range(NB):
                w = nb[b]
                pagg = pp.tile([P, P], f32, tag="agg")
                for k in range(w):
                    msg = lp.tile([P, P], f16, tag="msg")
                    nc.gpsimd.indirect_dma_start(
                        out=msg[:],
                        out_offset=None,
                        in_=x_full[:, :],
                        in_offset=bass.IndirectOffsetOnAxis(
                            ap=srci[:, col + k:col + k + 1], axis=0
                        ),
                    )
                    Pt = lp.tile([P, P], f16, tag="P")
                    nc.vector.tensor_scalar(
                        out=Pt[:], in0=iota_f[:],
                        scalar1=ldstf[:, col + k:col + k + 1],
                        scalar2=rlf[:, col + k:col + k + 1],
                        op0=mybir.AluOpType.is_equal,
                        op1=mybir.AluOpType.mult,
                    )
                    nc.tensor.matmul(
                        out=pagg[:], lhsT=msg[:], rhs=Pt[:],
                        start=(k == 0), stop=(k == w - 1),
                    )
                aggT = lp.tile([P, P], f16, tag="aggT")
                nc.vector.tensor_copy(out=aggT[:], in_=pagg[:])
                for j in range(2):
                    ph = php.tile([P, P], f32, tag="h")
                    nc.tensor.matmul(
                        out=ph[:], lhsT=wp_sb[:, j * P:(j + 1) * P],
                        rhs=aggT[:], start=True, stop=False,
                    )
                    nc.tensor.matmul(
                        out=ph[:], lhsT=wp_sb[:, HID + j * P:HID + (j + 1) * P],
                        rhs=xT[:, b * P:(b + 1) * P], start=False, stop=True,
                    )
                    nc.scalar.activation(
                        out=hT[j][:, b * P:(b + 1) * P], in_=ph[:],
                        func=mybir.ActivationFunctionType.Relu,
                        bias=b1f[:, j:j + 1],
                    )
                pzo = php.tile([P, 4], f32, tag="zo")
                for j in range(2):
                    nc.tensor.matmul(
                        out=pzo[:], lhsT=hT[j][:, b * P:(b + 1) * P],
                        rhs=wp_sb[:, 2 * HID + 4 * j:2 * HID + 4 * j + 4],
                        start=(j == 0), stop=(j == 1),
                    )
                nc.vector.tensor_copy(out=z_sb[:, 2 * b:2 * b + 2], in_=pzo[:, 0:2])
                nc.vector.tensor_tensor(
                    out=o_sb[:, 2 * b:2 * b + 2], in0=pzo[:, 2:4], in1=b2f[:],
                    op=mybir.AluOpType.add,
                )
                nc.sync.dma_start(
                    out=z_own[b * P:(b + 1) * P, :], in_=z_sb[:, 2 * b:2 * b + 2]
                )
                col += w

            nc.gpsimd.collective_compute(
                "AllGather",
                mybir.AluOpType.bypass,
                replica_groups=[list(range(NCORES))],
                ins=[z_own[:, :]],
                outs=[z_full[:, :]],
            )

            # ---- layer 2 ----
            col = 0
            for b in range(NB):
                w = nb[b]
                pa2 = pp.tile([P, 2], f32, tag="agg", name="pa2")
                for k in range(w):
                    zg = lp.tile([P, 2], f32, tag="zg")
                    nc.gpsimd.indirect_dma_start(
                        out=zg[:],
                        out_offset=None,
                        in_=z_full[:, :],
                        in_offset=bass.IndirectOffsetOnAxis(
                            ap=srci[:, col + k:col + k + 1], axis=0
                        ),
                    )
                    P2 = lp.tile([P, P], f32, tag="P2")
                    nc.vector.tensor_scalar(
                        out=P2[:], in0=iota_f[:],
                        scalar1=ldstf[:, col + k:col + k + 1],
                        scalar2=rlf[:, col + k:col + k + 1],
                        op0=mybir.AluOpType.is_equal,
                        op1=mybir.AluOpType.mult,
                    )
                    nc.tensor.matmul(
                        out=pa2[:], lhsT=P2[:], rhs=zg[:],
                        start=(k == 0), stop=(k == w - 1),
                    )
                nc.vector.tensor_tensor(
                    out=out_sb[:, 2 * b:2 * b + 2], in0=pa2[:],
                    in1=o_sb[:, 2 * b:2 * b + 2], op=mybir.AluOpType.add,
                )
                col += w

            nc.sync.dma_start(out=out_d[:, :], in_=out_sb[:])
    nc.compile()
    return nc


def _run(inputs, repeat=1):
    in_maps, nb = _host_prep(**inputs)
    nc = _build(nb)
    trace = bool(os.environ.get("TRACE"))
    best = None
    res = None
    for _ in range(repeat):
        t0 = time.perf_counter()
        res = run_bass_kernel_spmd(
            nc, [dict(m) for m in in_maps], core_ids=list(range(NCORES)),
            trace=trace,
        )
        dt = time.perf_counter() - t0
        best = dt if best is None else min(best, dt)
    if trace and res.exec_time_ns is not None:
        print(f"device exec_time: {res.exec_time_ns} ns")
    outs = []
    for c in range(NCORES):
        a = res.results[c]["out"]  # [128, 98]
        outs.append(a.reshape(P, NB, 2).transpose(1, 0, 2).reshape(ROWS, 2))
    full = np.concatenate(outs, axis=0)[:N]
    return full.astype(np.float32), best


def kernel(**inputs):
    out, _ = _run(inputs, repeat=1)
    return out


# revision 6
# speedup vs baseline: 5.6257x; 1.0574x over previous
"""GraphSAGE 2-layer fraud detector on 8 trn2 NeuronCores.

Strategy (dst-partitioned, DMA scatter-accumulate; wire+instruction optimized):
  - Host->device wire: inputs minimized and split into parallel streams
    (x fp16 sharded by node rows + feature-major copy, u16 edge table,
    fp16 packed weights). x is AllGathered on-device (NeuronLink).
  - Aggregation uses indirect-DMA gather with compute_op=add: the host
    assigns each edge of dst-block b to (chunk k, partition = local dst).
    Chunk 0 overwrites (bypass), later chunks accumulate, so
    agg[d, :] = sum_k x_full[srcq[d, k], :] with zero per-edge compute-engine
    work. Pad slots point at an all-zero x row. Chunks per block = max
    in-block degree (equalized across cores).
  - Per block: scale agg by 1/deg (tensor_scalar), PE-transpose to
    feature-major, then h = relu(W1l@aggT + W1r@xT + b1) in 512-wide
    windows; z = h@W2l.T, o = h@W2r.T + b2 per block.
  - z written to DRAM in natural node order (single strided DMA),
    AllGathered (50KB/core); layer 2 reuses the SAME srcq table to
    gather-accumulate the 2-wide z into agg2; out = agg2/deg + o.
"""

import os
import time

import numpy as np

import concourse.bass as bass
import concourse.mybir as mybir
import concourse.tile as tile
from concourse import bacc
from concourse.bass_utils import run_bass_kernel_spmd

N = 50000
E = 800000
IN_C = 128
HID = 256
OUT_C = 2
NCORES = 8
P = 128
NB = 49                 # dst blocks per core
ROWS = NB * P           # 6272 rows per core
NP = NCORES * ROWS      # 50176 padded nodes
NXS = 8                 # x shard splits (wire streams)
XSR = ROWS // NXS       # 784 rows per split
WIN = 4                 # dst blocks per h-matmul window
NW = (NB + WIN - 1) // WIN  # 13 windows (last is 1 block)

f32 = mybir.dt.float32
f16 = mybir.dt.float16
i32 = mybir.dt.int32
u16 = mybir.dt.uint16


def _host_prep(x, edge_index, W1l, b1, W1r, W2l, b2, W2r):
    src = np.asarray(edge_index[0]).astype(np.int64)
    dst = np.asarray(edge_index[1]).astype(np.int64)
    cnt = np.bincount(dst, minlength=NP)
    recip = (1.0 / np.maximum(cnt, 1)).astype(np.float32)

    # order edges by dst; per dst, edges are consecutive
    order = np.argsort(dst, kind="stable")
    s_src = src[order]
    starts = np.concatenate([[0], np.cumsum(cnt)])  # [NP+1]

    # chunks per block position = max in-block degree across cores
    deg_blk = cnt.reshape(NCORES, NB, P)
    nbk = deg_blk.max(axis=(0, 2))          # [NB]
    nbk = np.maximum(nbk, 1)
    C1 = int(nbk.sum())

    srcq = np.full((NCORES, P, C1), N, dtype=np.uint16)  # pad -> zero row
    col = 0
    for b in range(NB):
        w = int(nbk[b])
        for c in range(NCORES):
            base = (c * NB + b) * P
            for d in range(P):
                n0 = base + d
                k = int(cnt[n0])
                if k:
                    srcq[c, d, col:col + k] = s_src[starts[n0]:starts[n0] + k]
        col += w
    c1a = C1 // 2

    x_pad = np.zeros((NP, IN_C), np.float16)
    x_pad[:N] = np.asarray(x, dtype=np.float32).astype(np.float16)

    # packed fp16 weights: W1lT | W1rT | Wzo | b1p | b2b(tiled 4 blocks)
    WP = 2 * HID + 8 + 2 + 8
    wpack = np.zeros((P, WP), np.float16)
    wpack[:, 0:HID] = np.asarray(W1l).T.astype(np.float16)
    wpack[:, HID:2 * HID] = np.asarray(W1r).T.astype(np.float16)
    for j in range(2):
        wpack[:, 2 * HID + 4 * j:2 * HID + 4 * j + 2] = \
            np.asarray(W2l).T[j * P:(j + 1) * P, :].astype(np.float16)
        wpack[:, 2 * HID + 4 * j + 2:2 * HID + 4 * j + 4] = \
            np.asarray(W2r).T[j * P:(j + 1) * P, :].astype(np.float16)
    wpack[:, 2 * HID + 8:2 * HID + 10] = \
        np.asarray(b1).reshape(2, P).T.astype(np.float16)
    wpack[:, 2 * HID + 10:2 * HID + 18] = \
        np.tile(np.asarray(b2).reshape(1, 2), (P, 4)).astype(np.float16)

    recip_c = recip.reshape(NCORES, NB, P).transpose(0, 2, 1).copy()  # [c,P,NB]

    in_maps = []
    for c in range(NCORES):
        xc = x_pad[c * ROWS:(c + 1) * ROWS]
        xcT = np.ascontiguousarray(xc.T)  # [128, 6272] f16
        m = {
            "srcA": np.ascontiguousarray(srcq[c, :, :c1a]),
            "srcB": np.ascontiguousarray(srcq[c, :, c1a:]),
            "wpack": wpack,
            "recip": np.ascontiguousarray(recip_c[c]),
        }
        for s in range(NXS):
            m[f"xs{s}"] = np.ascontiguousarray(xc[s * XSR:(s + 1) * XSR])
            m[f"xt{s}"] = np.ascontiguousarray(xcT[:, s * XSR:(s + 1) * XSR])
        in_maps.append(m)
    return in_maps, [int(v) for v in nbk]


def _build(nbk):
    C1 = sum(nbk)
    c1a = C1 // 2
    WP = 2 * HID + 8 + 2 + 8
    nc = bacc.Bacc(None, target_bir_lowering=False, debug=False)

    xs_d = [
        nc.dram_tensor(f"xs{s}", [XSR, IN_C], f16, kind="ExternalInput")
        for s in range(NXS)
    ]
    xt_d = [
        nc.dram_tensor(f"xt{s}", [P, XSR], f16, kind="ExternalInput")
        for s in range(NXS)
    ]
    srcA_d = nc.dram_tensor("srcA", [P, c1a], u16, kind="ExternalInput")
    srcB_d = nc.dram_tensor("srcB", [P, C1 - c1a], u16, kind="ExternalInput")
    wpack_d = nc.dram_tensor("wpack", [P, WP], f16, kind="ExternalInput")
    recip_d = nc.dram_tensor("recip", [P, NB], f32, kind="ExternalInput")
    out_d = nc.dram_tensor("out", [P, 2 * NB], f32, kind="ExternalOutput")

    with tile.TileContext(nc) as tc:
        with (
            tc.tile_pool(name="big", bufs=1) as big,
            tc.tile_pool(name="lp", bufs=4) as lp,
            tc.tile_pool(name="pp", bufs=2, space="PSUM") as pp,
            tc.tile_pool(name="php", bufs=2, space="PSUM") as php,
            tc.tile_pool(name="dram", bufs=1, space="DRAM") as dp,
        ):
            # ---- input staging ----
            srcu = big.tile([P, C1], u16, tag="srcu")
            nc.sync.dma_start(out=srcu[:, :c1a], in_=srcA_d[:, :])
            nc.sync.dma_start(out=srcu[:, c1a:], in_=srcB_d[:, :])
            wp_sb = big.tile([P, WP], f16, tag="wp")
            nc.sync.dma_start(out=wp_sb[:], in_=wpack_d[:, :])
            recip_sb = big.tile([P, NB], f32, tag="recip")
            nc.sync.dma_start(out=recip_sb[:], in_=recip_d[:, :])

            x_own = dp.tile([ROWS, IN_C], f16, tag="xown")
            x_full = dp.tile([NP, IN_C], f16, tag="xfull")
            xT = big.tile([P, ROWS], f16, tag="xT")
            for s in range(NXS):
                nc.sync.dma_start(
                    out=x_own[s * XSR:(s + 1) * XSR, :], in_=xs_d[s][:, :]
                )
                nc.sync.dma_start(
                    out=xT[:, s * XSR:(s + 1) * XSR], in_=xt_d[s][:, :]
                )
            nc.gpsimd.collective_compute(
                "AllGather",
                mybir.AluOpType.bypass,
                replica_groups=[list(range(NCORES))],
                ins=[x_own[:, :]],
                outs=[x_full[:, :]],
            )

            srci = big.tile([P, C1], i32, tag="srci")
            nc.vector.tensor_copy(out=srci[:], in_=srcu[:])
            b1f = big.tile([P, 2], f32, tag="b1f")
            nc.vector.tensor_copy(out=b1f[:], in_=wp_sb[:, 2 * HID + 8:2 * HID + 10])
            b2f = big.tile([P, 2], f32, tag="b2f")
            nc.vector.tensor_copy(out=b2f[:], in_=wp_sb[:, 2 * HID + 10:2 * HID + 12])

            # identity (f16) for PE transposes
            iota_i = big.tile([P, P], i32, tag="iotai")
            nc.gpsimd.iota(out=iota_i[:], pattern=[[1, P]], base=0,
                           channel_multiplier=0)
            iotap_i = big.tile([P, 1], i32, tag="iotapi")
            nc.gpsimd.iota(out=iotap_i[:], pattern=[[0, 1]], base=0,
                           channel_multiplier=1)
            iota_f = big.tile([P, P], f32, tag="iotaf")
            nc.vector.tensor_copy(out=iota_f[:], in_=iota_i[:])
            iotap_f = big.tile([P, 1], f32, tag="iotapf")
            nc.vector.tensor_copy(out=iotap_f[:], in_=iotap_i[:])
            ident16 = big.tile([P, P], f16, tag="ident16")
            nc.vector.tensor_scalar(
                out=ident16[:], in0=iota_f[:], scalar1=iotap_f[:, 0:1],
                scalar2=None, op0=mybir.AluOpType.is_equal,
            )

            aggT_all = big.tile([P, ROWS], f16, tag="aggT")
            hT = [
                big.tile([P, ROWS], f16, tag=f"hT{j}", name=f"hT{j}")
                for j in range(2)
            ]
            z_sb = big.tile([P, 2 * NB], f32, tag="z")
            o_sb = big.tile([P, 2 * NB], f32, tag="o")
            out_sb = big.tile([P, 2 * NB], f32, tag="outs")
            z_own = dp.tile([ROWS, 2], f32, tag="zown")
            z_full = dp.tile([NP, 2], f32, tag="zfull")

            # ---- layer 1 aggregation: gather-accumulate per dst block ----
            col = 0
            for b in range(NB):
                w = nbk[b]
                agg = lp.tile([P, P], f32, tag="agg")
                for k in range(w):
                    nc.gpsimd.indirect_dma_start(
                        out=agg[:],
                        out_offset=None,
                        in_=x_full[:, :],
                        in_offset=bass.IndirectOffsetOnAxis(
                            ap=srci[:, col + k:col + k + 1], axis=0
                        ),
                        compute_op=(mybir.AluOpType.bypass if k == 0
                                    else mybir.AluOpType.add),
                    )
                aggm = lp.tile([P, P], f16, tag="aggm")
                nc.vector.tensor_scalar(
                    out=aggm[:], in0=agg[:], scalar1=recip_sb[:, b:b + 1],
                    scalar2=None, op0=mybir.AluOpType.mult,
                )
                ptr = pp.tile([P, P], f16, tag="tr")
                nc.tensor.transpose(out=ptr[:], in_=aggm[:], identity=ident16[:])
                nc.vector.tensor_copy(
                    out=aggT_all[:, b * P:(b + 1) * P], in_=ptr[:]
                )
                col += w

            # ---- layer 1 dense part, in 512-wide windows ----
            for wi in range(NW):
                lo = wi * WIN * P
                hi = min(ROWS, lo + WIN * P)
                for j in range(2):
                    ph = php.tile([P, hi - lo], f32, tag="h")
                    nc.tensor.matmul(
                        out=ph[:], lhsT=wp_sb[:, j * P:(j + 1) * P],
                        rhs=aggT_all[:, lo:hi], start=True, stop=False,
                    )
                    nc.tensor.matmul(
                        out=ph[:], lhsT=wp_sb[:, HID + j * P:HID + (j + 1) * P],
                        rhs=xT[:, lo:hi], start=False, stop=True,
                    )
                    nc.scalar.activation(
                        out=hT[j][:, lo:hi], in_=ph[:],
                        func=mybir.ActivationFunctionType.Relu,
                        bias=b1f[:, j:j + 1],
                    )

            # ---- z/o per block ----
            for b in range(NB):
                pzo = php.tile([P, 4], f32, tag="zo")
                for j in range(2):
                    nc.tensor.matmul(
                        out=pzo[:], lhsT=hT[j][:, b * P:(b + 1) * P],
                        rhs=wp_sb[:, 2 * HID + 4 * j:2 * HID + 4 * j + 4],
                        start=(j == 0), stop=(j == 1),
                    )
                nc.vector.tensor_copy(out=z_sb[:, 2 * b:2 * b + 2], in_=pzo[:, 0:2])
                nc.vector.tensor_tensor(
                    out=o_sb[:, 2 * b:2 * b + 2], in0=pzo[:, 2:4],
                    in1=b2f[:], op=mybir.AluOpType.add,
                )

            # z -> DRAM natural node order (single strided DMA), allgather
            nc.sync.dma_start(
                out=z_own[:, :].rearrange("(b p) j -> p b j", p=P),
                in_=z_sb[:, :].rearrange("p (b j) -> p b j", j=2),
            )
            nc.gpsimd.collective_compute(
                "AllGather",
                mybir.AluOpType.bypass,
                replica_groups=[list(range(NCORES))],
                ins=[z_own[:, :]],
                outs=[z_full[:, :]],
            )

            # ---- layer 2: gather-accumulate z, same table ----
            col = 0
            for b in range(NB):
                w = nbk[b]
                agg2 = lp.tile([P, 2], f32, tag="agg2")
                for k in range(w):
                    nc.gpsimd.indirect_dma_start(
                        out=agg2[:],
                        out_offset=None,
                        in_=z_full[:, :],
                        in_offset=bass.IndirectOffsetOnAxis(
                            ap=srci[:, col + k:col + k + 1], axis=0
                        ),
                        compute_op=(mybir.AluOpType.bypass if k == 0
                                    else mybir.AluOpType.add),
                    )
                red2 = lp.tile([P, 2], f32, tag="red2")
                nc.vector.tensor_scalar(
                    out=red2[:], in0=agg2[:], scalar1=recip_sb[:, b:b + 1],
                    scalar2=None, op0=mybir.AluOpType.mult,
                )
                nc.vector.tensor_tensor(
                    out=out_sb[:, 2 * b:2 * b + 2], in0=red2[:],
                    in1=o_sb[:, 2 * b:2 * b + 2], op=mybir.AluOpType.add,
                )
                col += w

            nc.sync.dma_start(out=out_d[:, :], in_=out_sb[:])
    nc.compile()
    return nc


def _run(inputs, repeat=1):
    in_maps, nbk = _host_prep(**inputs)
    nc = _build(nbk)
    best = None
    res = None
    for _ in range(repeat):
        t0 = time.perf_counter()
        res = run_bass_kernel_spmd(
            nc, [dict(m) for m in in_maps], core_ids=list(range(NCORES))
        )
        dt = time.perf_counter() - t0
        best = dt if best is None else min(best, dt)
    outs = []
    for c in range(NCORES):
        a = res.results[c]["out"]  # [128, 98]
        outs.append(a.reshape(P, NB, 2).transpose(1, 0, 2).reshape(ROWS, 2))
    full = np.concatenate(outs, axis=0)[:N]
    return full.astype(np.float32), best


def kernel(**inputs):
    out, _ = _run(inputs, repeat=1)
    return out


# revision 14
# speedup vs baseline: 10.1046x; 1.7961x over previous
"""GraphSAGE 2-layer fraud detector on 8 trn2 NeuronCores.

Strategy (dst-partitioned, DMA scatter-accumulate; wire+instruction optimized):
  - Host->device wire dominates and is serial (~12ms/MB + per-tensor fixed
    cost), so inputs are minimal and few: one fp16 x shard per core, one u16
    edge/permutation table, one packed fp16 weight tensor (recip included).
    x is AllGathered on-device.
  - Aggregation uses indirect-DMA gather with compute_op=add: each edge of
    dst-block b is assigned to (chunk k, partition = local dst position).
    Chunk 0 overwrites (bypass), later chunks accumulate, so
    agg[d, :] = sum_k x_full[srcq[d, k], :] with zero per-edge compute-engine
    work. Pad slots point at an all-zero x row.
  - Each core's dst nodes are SORTED BY IN-DEGREE before blocking, so the
    chunk count per block (= max in-block degree) tracks the block's degree
    quantile instead of the global max: ~820 chunks/layer instead of ~1470.
    All node-order-dependent state (recip, xT, z rows, out rows) follows the
    permutation; z is scattered back to natural node order via indirect DMA
    so layer 2 can gather by global node id, and the host inverse-permutes
    the final output.
  - Per block: scale agg by 1/deg, PE-transpose to feature-major; xT built
    by indirect-gathering the permuted rows from x_full + PE transpose.
    h = relu(W1l@aggT + W1r@xT + b1) in 512-wide windows; z = h@W2l.T,
    o = h@W2r.T + b2 per block. z AllGathered (50KB/core); layer 2 reuses
    the SAME srcq table to gather-accumulate the 2-wide z; out = agg2/deg+o.
"""

import os
import time

os.environ.setdefault("JAX_PLATFORMS", "cpu,axon")
os.environ.setdefault("NEURON_RT_RESET_CORES", "1")

import numpy as np

import concourse.bass as bass
import concourse.mybir as mybir
import concourse.tile as tile
from concourse import bacc
from concourse.bass_utils import run_bass_kernel_spmd

N = 50000
E = 800000
IN_C = 128
HID = 256
OUT_C = 2
NCORES = 8
P = 128
NB = 49                 # dst blocks per core
ROWS = NB * P           # 6272 rows per core
NP = NCORES * ROWS      # 50176 padded nodes
WIN = 4                 # dst blocks per h-matmul window
NW = (NB + WIN - 1) // WIN  # 13 windows (last is 1 block)

f32 = mybir.dt.float32
f16 = mybir.dt.float16
i32 = mybir.dt.int32
u16 = mybir.dt.uint16

WP = 2 * HID + 8 + 2 + 2 + NB  # W1lT | W1rT | Wzo | b1p | b2b | recip


def _host_prep(x, edge_index, W1l, b1, W1r, W2l, b2, W2r):
    src = np.asarray(edge_index[0]).astype(np.int64)
    dst = np.asarray(edge_index[1]).astype(np.int64)
    cnt = np.bincount(dst, minlength=NP)
    recip = (1.0 / np.maximum(cnt, 1)).astype(np.float32)

    order = np.argsort(dst, kind="stable")
    s_src = src[order]
    starts = np.concatenate([[0], np.cumsum(cnt)])  # [NP+1]

    # per-core permutation: dsts sorted by in-degree (desc)
    cnt_c = cnt.reshape(NCORES, ROWS)
    perm = np.argsort(-cnt_c, axis=1, kind="stable")      # [c, pos] -> local dst
    pdeg = np.take_along_axis(cnt_c, perm, axis=1)        # degree at position
    nbk = np.maximum(pdeg.reshape(NCORES, NB, P).max(axis=2).max(axis=0), 1)
    C1 = int(nbk.sum())
    CT = C1 + 2 * NB     # + xT perm cols + z scatter cols

    srcq = np.full((NCORES, P, CT), N, dtype=np.uint16)
    for c in range(NCORES):
        col = 0
        for b in range(NB):
            w = int(nbk[b])
            for d in range(P):
                loc = int(perm[c, b * P + d])
                n0 = c * ROWS + loc
                k = int(cnt[n0])
                if k:
                    srcq[c, d, col:col + k] = s_src[starts[n0]:starts[n0] + k]
            col += w
        # xT gather cols: global x row of permuted position (b, d)
        srcq[c, :, C1:C1 + NB] = (c * ROWS + perm[c]).reshape(NB, P).T
        # z scatter cols: natural local row for permuted position (b, d)
        srcq[c, :, C1 + NB:] = perm[c].reshape(NB, P).T

    x_pad = np.zeros((NP, IN_C), np.float16)
    x_pad[:N] = np.asarray(x, dtype=np.float32).astype(np.float16)

    wpack = np.zeros((P, WP), np.float16)
    wpack[:, 0:HID] = np.asarray(W1l).T.astype(np.float16)
    wpack[:, HID:2 * HID] = np.asarray(W1r).T.astype(np.float16)
    for j in range(2):
        wpack[:, 2 * HID + 4 * j:2 * HID + 4 * j + 2] = \
            np.asarray(W2l).T[j * P:(j + 1) * P, :].astype(np.float16)
        wpack[:, 2 * HID + 4 * j + 2:2 * HID + 4 * j + 4] = \
            np.asarray(W2r).T[j * P:(j + 1) * P, :].astype(np.float16)
    wpack[:, 2 * HID + 8:2 * HID + 10] = \
        np.asarray(b1).reshape(2, P).T.astype(np.float16)
    wpack[:, 2 * HID + 10:2 * HID + 12] = \
        np.tile(np.asarray(b2).reshape(1, 2), (P, 1)).astype(np.float16)

    in_maps = []
    for c in range(NCORES):
        wpc = wpack.copy()
        rc = recip[c * ROWS:(c + 1) * ROWS][perm[c]]      # permuted recip
        wpc[:, 2 * HID + 12:] = rc.reshape(NB, P).T.astype(np.float16)
        m = {
            "srcq": np.ascontiguousarray(srcq[c]),
            "wpack": wpc,
            "xs": np.ascontiguousarray(x_pad[c * ROWS:(c + 1) * ROWS]),
        }
        in_maps.append(m)
    return in_maps, [int(v) for v in nbk], perm


def _build(nbk):
    C1 = sum(nbk)
    CT = C1 + 2 * NB
    nc = bacc.Bacc(None, target_bir_lowering=False, debug=False)

    xs_d = nc.dram_tensor("xs", [ROWS, IN_C], f16, kind="ExternalInput")
    srcq_d = nc.dram_tensor("srcq", [P, CT], u16, kind="ExternalInput")
    wpack_d = nc.dram_tensor("wpack", [P, WP], f16, kind="ExternalInput")
    out_d = nc.dram_tensor("out", [P, 2 * NB], f32, kind="ExternalOutput")

    with tile.TileContext(nc) as tc:
        with (
            tc.tile_pool(name="big", bufs=1) as big,
            tc.tile_pool(name="lp", bufs=4) as lp,
            tc.tile_pool(name="pp", bufs=2, space="PSUM") as pp,
            tc.tile_pool(name="php", bufs=2, space="PSUM") as php,
            tc.tile_pool(name="dram", bufs=1, space="DRAM") as dp,
        ):
            # ---- input staging ----
            srcu = big.tile([P, CT], u16, tag="srcu")
            nc.sync.dma_start(out=srcu[:], in_=srcq_d[:, :])
            wp_sb = big.tile([P, WP], f16, tag="wp")
            nc.sync.dma_start(out=wp_sb[:], in_=wpack_d[:, :])

            x_own = dp.tile([ROWS, IN_C], f16, tag="xown")
            nc.sync.dma_start(out=x_own[:, :], in_=xs_d[:, :])
            x_full = dp.tile([NP, IN_C], f16, tag="xfull")
            nc.gpsimd.collective_compute(
                "AllGather",
                mybir.AluOpType.bypass,
                replica_groups=[list(range(NCORES))],
                ins=[x_own[:, :]],
                outs=[x_full[:, :]],
            )

            srci = big.tile([P, CT], i32, tag="srci")
            nc.vector.tensor_copy(out=srci[:], in_=srcu[:])
            b1f = big.tile([P, 2], f32, tag="b1f")
            nc.vector.tensor_copy(out=b1f[:], in_=wp_sb[:, 2 * HID + 8:2 * HID + 10])
            b2f = big.tile([P, 2], f32, tag="b2f")
            nc.vector.tensor_copy(out=b2f[:], in_=wp_sb[:, 2 * HID + 10:2 * HID + 12])
            recipf = big.tile([P, NB], f32, tag="recipf")
            nc.vector.tensor_copy(out=recipf[:], in_=wp_sb[:, 2 * HID + 12:])

            # identity (f16) for PE transposes
            iota_i = big.tile([P, P], i32, tag="iotai")
            nc.gpsimd.iota(out=iota_i[:], pattern=[[1, P]], base=0,
                           channel_multiplier=0)
            iotap_i = big.tile([P, 1], i32, tag="iotapi")
            nc.gpsimd.iota(out=iotap_i[:], pattern=[[0, 1]], base=0,
                           channel_multiplier=1)
            iota_f = big.tile([P, P], f32, tag="iotaf")
            nc.vector.tensor_copy(out=iota_f[:], in_=iota_i[:])
            iotap_f = big.tile([P, 1], f32, tag="iotapf")
            nc.vector.tensor_copy(out=iotap_f[:], in_=iotap_i[:])
            ident16 = big.tile([P, P], f16, tag="ident16")
            nc.vector.tensor_scalar(
                out=ident16[:], in0=iota_f[:], scalar1=iotap_f[:, 0:1],
                scalar2=None, op0=mybir.AluOpType.is_equal,
            )

            # xT: feature-major permuted own x (gather from x_full + transpose)
            xT = big.tile([P, ROWS], f16, tag="xT")
            for b in range(NB):
                xg = lp.tile([P, P], f16, tag="xg")
                nc.gpsimd.indirect_dma_start(
                    out=xg[:], out_offset=None, in_=x_full[:, :],
                    in_offset=bass.IndirectOffsetOnAxis(
                        ap=srci[:, C1 + b:C1 + b + 1], axis=0
                    ),
                )
                ptx = pp.tile([P, P], f16, tag="tr", name=f"ptx{b}")
                nc.tensor.transpose(out=ptx[:], in_=xg[:], identity=ident16[:])
                nc.vector.tensor_copy(out=xT[:, b * P:(b + 1) * P], in_=ptx[:])

            aggT_all = big.tile([P, ROWS], f16, tag="aggT")
            hT = [
                big.tile([P, ROWS], f16, tag=f"hT{j}", name=f"hT{j}")
                for j in range(2)
            ]
            z_sb = big.tile([P, 2 * NB], f32, tag="z")
            o_sb = big.tile([P, 2 * NB], f32, tag="o")
            out_sb = big.tile([P, 2 * NB], f32, tag="outs")
            z_own = dp.tile([ROWS, 2], f32, tag="zown")
            z_full = dp.tile([NP, 2], f32, tag="zfull")

            # ---- layer 1 aggregation: gather-accumulate per dst block ----
            col = 0
            for b in range(NB):
                w = nbk[b]
                agg = lp.tile([P, P], f16, tag="agg")
                for k in range(w):
                    nc.gpsimd.indirect_dma_start(
                        out=agg[:],
                        out_offset=None,
                        in_=x_full[:, :],
                        in_offset=bass.IndirectOffsetOnAxis(
                            ap=srci[:, col + k:col + k + 1], axis=0
                        ),
                        compute_op=(mybir.AluOpType.bypass if k == 0
                                    else mybir.AluOpType.add),
                    )
                aggm = lp.tile([P, P], f16, tag="aggm")
                nc.vector.tensor_scalar(
                    out=aggm[:], in0=agg[:], scalar1=recipf[:, b:b + 1],
                    scalar2=None, op0=mybir.AluOpType.mult,
                )
                ptr = pp.tile([P, P], f16, tag="tr")
                nc.tensor.transpose(out=ptr[:], in_=aggm[:], identity=ident16[:])
                nc.vector.tensor_copy(
                    out=aggT_all[:, b * P:(b + 1) * P], in_=ptr[:]
                )
                col += w

            # ---- layer 1 dense part, in 512-wide windows ----
            for wi in range(NW):
                lo = wi * WIN * P
                hi = min(ROWS, lo + WIN * P)
                for j in range(2):
                    ph = php.tile([P, hi - lo], f32, tag="h")
                    nc.tensor.matmul(
                        out=ph[:], lhsT=wp_sb[:, j * P:(j + 1) * P],
                        rhs=aggT_all[:, lo:hi], start=True, stop=False,
                    )
                    nc.tensor.matmul(
                        out=ph[:], lhsT=wp_sb[:, HID + j * P:HID + (j + 1) * P],
                        rhs=xT[:, lo:hi], start=False, stop=True,
                    )
                    nc.scalar.activation(
                        out=hT[j][:, lo:hi], in_=ph[:],
                        func=mybir.ActivationFunctionType.Relu,
                        bias=b1f[:, j:j + 1],
                    )

            # ---- z/o per block; z scattered to natural node order ----
            for b in range(NB):
                pzo = php.tile([P, 4], f32, tag="zo")
                for j in range(2):
                    nc.tensor.matmul(
                        out=pzo[:], lhsT=hT[j][:, b * P:(b + 1) * P],
                        rhs=wp_sb[:, 2 * HID + 4 * j:2 * HID + 4 * j + 4],
                        start=(j == 0), stop=(j == 1),
                    )
                nc.vector.tensor_copy(out=z_sb[:, 2 * b:2 * b + 2], in_=pzo[:, 0:2])
                nc.vector.tensor_tensor(
                    out=o_sb[:, 2 * b:2 * b + 2], in0=pzo[:, 2:4],
                    in1=b2f[:], op=mybir.AluOpType.add,
                )
                nc.gpsimd.indirect_dma_start(
                    out=z_own[:, :],
                    out_offset=bass.IndirectOffsetOnAxis(
                        ap=srci[:, C1 + NB + b:C1 + NB + b + 1], axis=0
                    ),
                    in_=z_sb[:, 2 * b:2 * b + 2],
                    in_offset=None,
                )

            nc.gpsimd.collective_compute(
                "AllGather",
                mybir.AluOpType.bypass,
                replica_groups=[list(range(NCORES))],
                ins=[z_own[:, :]],
                outs=[z_full[:, :]],
            )

            # ---- layer 2: gather-accumulate z, same table ----
            col = 0
            for b in range(NB):
                w = nbk[b]
                agg2 = lp.tile([P, 2], f32, tag="agg2")
                for k in range(w):
                    nc.gpsimd.indirect_dma_start(
                        out=agg2[:],
                        out_offset=None,
                        in_=z_full[:, :],
                        in_offset=bass.IndirectOffsetOnAxis(
                            ap=srci[:, col + k:col + k + 1], axis=0
                        ),
                        compute_op=(mybir.AluOpType.bypass if k == 0
                                    else mybir.AluOpType.add),
                    )
                red2 = lp.tile([P, 2], f32, tag="red2")
                nc.vector.tensor_scalar(
                    out=red2[:], in0=agg2[:], scalar1=recipf[:, b:b + 1],
                    scalar2=None, op0=mybir.AluOpType.mult,
                )
                nc.vector.tensor_tensor(
                    out=out_sb[:, 2 * b:2 * b + 2], in0=red2[:],
                    in1=o_sb[:, 2 * b:2 * b + 2], op=mybir.AluOpType.add,
                )
                col += w

            nc.sync.dma_start(out=out_d[:, :], in_=out_sb[:])
    nc.compile()
    return nc


def _run(inputs, repeat=1):
    in_maps, nbk, perm = _host_prep(**inputs)
    nc = _build(nbk)
    best = None
    res = None
    for _ in range(repeat):
        t0 = time.perf_counter()
        res = run_bass_kernel_spmd(
            nc, [dict(m) for m in in_maps], core_ids=list(range(NCORES))
        )
        dt = time.perf_counter() - t0
        best = dt if best is None else min(best, dt)
    full = np.empty((NP, 2), np.float32)
    for c in range(NCORES):
        a = res.results[c]["out"]  # [128, 98] in permuted order
        ap = a.reshape(P, NB, 2).transpose(1, 0, 2).reshape(ROWS, 2)
        full[c * ROWS + perm[c]] = ap
    return full[:N].astype(np.float32), best


def kernel(**inputs):
    out, _ = _run(inputs, repeat=1)
    return out


# revision 15
# speedup vs baseline: 10.1136x; 1.0009x over previous
"""GraphSAGE 2-layer fraud detector on 8 trn2 NeuronCores.

Strategy (dst-partitioned, DMA scatter-accumulate; wire+instruction optimized):
  - Host->device wire dominates and is serial (~12ms/MB + per-tensor fixed
    cost), so inputs are minimal and few: one fp16 x shard per core, one u16
    edge/permutation table, one packed fp16 weight tensor (recip included).
    x is AllGathered on-device.
  - Aggregation uses indirect-DMA gather with compute_op=add: each edge of
    dst-block b is assigned to (chunk k, partition = local dst position).
    Chunk 0 overwrites (bypass), later chunks accumulate, so
    agg[d, :] = sum_k x_full[srcq[d, k], :] with zero per-edge compute-engine
    work. Pad slots point at an all-zero x row.
  - Each core's dst nodes are SORTED BY IN-DEGREE before blocking, so the
    chunk count per block (= max in-block degree) tracks the block's degree
    quantile instead of the global max: ~820 chunks/layer instead of ~1470.
    All node-order-dependent state (recip, xT, z rows, out rows) follows the
    permutation; z is scattered back to natural node order via indirect DMA
    so layer 2 can gather by global node id, and the host inverse-permutes
    the final output.
  - Per block: scale agg by 1/deg, PE-transpose to feature-major; xT built
    by indirect-gathering the permuted rows from x_full + PE transpose.
    h = relu(W1l@aggT + W1r@xT + b1) in 512-wide windows; z = h@W2l.T,
    o = h@W2r.T + b2 per block. z AllGathered (50KB/core); layer 2 reuses
    the SAME srcq table to gather-accumulate the 2-wide z; out = agg2/deg+o.
"""

import os
import time

os.environ.setdefault("JAX_PLATFORMS", "cpu,axon")
os.environ.setdefault("NEURON_RT_RESET_CORES", "1")

import numpy as np

import concourse.bass as bass
import concourse.mybir as mybir
import concourse.tile as tile
from concourse import bacc
from concourse.bass_utils import run_bass_kernel_spmd

N = 50000
E = 800000
IN_C = 128
HID = 256
OUT_C = 2
NCORES = 8
P = 128
NB = 49                 # dst blocks per core
ROWS = NB * P           # 6272 rows per core
NP = NCORES * ROWS      # 50176 padded nodes
WIN = 4                 # dst blocks per h-matmul window
NW = (NB + WIN - 1) // WIN  # 13 windows (last is 1 block)

f32 = mybir.dt.float32
f16 = mybir.dt.float16
i32 = mybir.dt.int32
u16 = mybir.dt.uint16

WP = 2 * HID + 8 + 2 + 2 + NB  # W1lT | W1rT | Wzo | b1p | b2b | recip


def _host_prep(x, edge_index, W1l, b1, W1r, W2l, b2, W2r):
    src = np.asarray(edge_index[0]).astype(np.int64)
    dst = np.asarray(edge_index[1]).astype(np.int64)
    cnt = np.bincount(dst, minlength=NP)
    recip = (1.0 / np.maximum(cnt, 1)).astype(np.float32)

    order = np.argsort(dst, kind="stable")
    s_src = src[order]
    starts = np.concatenate([[0], np.cumsum(cnt)])  # [NP+1]

    # per-core permutation: dsts sorted by in-degree (desc)
    cnt_c = cnt.reshape(NCORES, ROWS)
    perm = np.argsort(-cnt_c, axis=1, kind="stable")      # [c, pos] -> local dst
    pdeg = np.take_along_axis(cnt_c, perm, axis=1)        # degree at position
    nbk = np.maximum(pdeg.reshape(NCORES, NB, P).max(axis=2).max(axis=0), 1)
    C1 = int(nbk.sum())
    CT = C1 + 2 * NB     # + xT perm cols + z scatter cols

    srcq = np.full((NCORES, P, CT), N, dtype=np.uint16)
    for c in range(NCORES):
        col = 0
        for b in range(NB):
            w = int(nbk[b])
            for d in range(P):
                loc = int(perm[c, b * P + d])
                n0 = c * ROWS + loc
                k = int(cnt[n0])
                if k:
                    srcq[c, d, col:col + k] = s_src[starts[n0]:starts[n0] + k]
            col += w
        # xT gather cols: global x row of permuted position (b, d)
        srcq[c, :, C1:C1 + NB] = (c * ROWS + perm[c]).reshape(NB, P).T
        # z scatter cols: natural local row for permuted position (b, d)
        srcq[c, :, C1 + NB:] = perm[c].reshape(NB, P).T

    x_pad = np.zeros((NP, IN_C), np.float16)
    x_pad[:N] = np.asarray(x, dtype=np.float32).astype(np.float16)

    wpack = np.zeros((P, WP), np.float16)
    wpack[:, 0:HID] = np.asarray(W1l).T.astype(np.float16)
    wpack[:, HID:2 * HID] = np.asarray(W1r).T.astype(np.float16)
    for j in range(2):
        wpack[:, 2 * HID + 4 * j:2 * HID + 4 * j + 2] = \
            np.asarray(W2l).T[j * P:(j + 1) * P, :].astype(np.float16)
        wpack[:, 2 * HID + 4 * j + 2:2 * HID + 4 * j + 4] = \
            np.asarray(W2r).T[j * P:(j + 1) * P, :].astype(np.float16)
    wpack[:, 2 * HID + 8:2 * HID + 10] = \
        np.asarray(b1).reshape(2, P).T.astype(np.float16)
    wpack[:, 2 * HID + 10:2 * HID + 12] = \
        np.tile(np.asarray(b2).reshape(1, 2), (P, 1)).astype(np.float16)

    in_maps = []
    for c in range(NCORES):
        wpc = wpack.copy()
        rc = recip[c * ROWS:(c + 1) * ROWS][perm[c]]      # permuted recip
        wpc[:, 2 * HID + 12:] = rc.reshape(NB, P).T.astype(np.float16)
        m = {
            "srcq": np.ascontiguousarray(srcq[c]),
            "wpack": wpc,
            "xs": np.ascontiguousarray(x_pad[c * ROWS:(c + 1) * ROWS]),
        }
        in_maps.append(m)
    return in_maps, [int(v) for v in nbk], perm


def _build(nbk):
    C1 = sum(nbk)
    CT = C1 + 2 * NB
    nc = bacc.Bacc(None, target_bir_lowering=False, debug=False)

    xs_d = nc.dram_tensor("xs", [ROWS, IN_C], f16, kind="ExternalInput")
    srcq_d = nc.dram_tensor("srcq", [P, CT], u16, kind="ExternalInput")
    wpack_d = nc.dram_tensor("wpack", [P, WP], f16, kind="ExternalInput")
    out_d = nc.dram_tensor("out", [P, 2 * NB], f32, kind="ExternalOutput")

    with tile.TileContext(nc) as tc:
        with (
            tc.tile_pool(name="big", bufs=1) as big,
            tc.tile_pool(name="lp", bufs=4) as lp,
            tc.tile_pool(name="pp", bufs=2, space="PSUM") as pp,
            tc.tile_pool(name="php", bufs=2, space="PSUM") as php,
            tc.tile_pool(name="dram", bufs=1, space="DRAM") as dp,
        ):
            # ---- input staging ----
            srcu = big.tile([P, CT], u16, tag="srcu")
            nc.sync.dma_start(out=srcu[:], in_=srcq_d[:, :])
            wp_sb = big.tile([P, WP], f16, tag="wp")
            nc.sync.dma_start(out=wp_sb[:], in_=wpack_d[:, :])

            x_own = dp.tile([ROWS, IN_C], f16, tag="xown")
            nc.sync.dma_start(out=x_own[:, :], in_=xs_d[:, :])
            x_full = dp.tile([NP, IN_C], f16, tag="xfull")
            nc.gpsimd.collective_compute(
                "AllGather",
                mybir.AluOpType.bypass,
                replica_groups=[list(range(NCORES))],
                ins=[x_own[:, :]],
                outs=[x_full[:, :]],
            )

            srci = big.tile([P, CT], i32, tag="srci")
            nc.vector.tensor_copy(out=srci[:], in_=srcu[:])
            b1f = big.tile([P, 2], f32, tag="b1f")
            nc.vector.tensor_copy(out=b1f[:], in_=wp_sb[:, 2 * HID + 8:2 * HID + 10])
            b2f = big.tile([P, 2], f32, tag="b2f")
            nc.vector.tensor_copy(out=b2f[:], in_=wp_sb[:, 2 * HID + 10:2 * HID + 12])
            recipf = big.tile([P, NB], f32, tag="recipf")
            nc.vector.tensor_copy(out=recipf[:], in_=wp_sb[:, 2 * HID + 12:])

            # identity (f16) for PE transposes
            iota_i = big.tile([P, P], i32, tag="iotai")
            nc.gpsimd.iota(out=iota_i[:], pattern=[[1, P]], base=0,
                           channel_multiplier=0)
            iotap_i = big.tile([P, 1], i32, tag="iotapi")
            nc.gpsimd.iota(out=iotap_i[:], pattern=[[0, 1]], base=0,
                           channel_multiplier=1)
            iota_f = big.tile([P, P], f32, tag="iotaf")
            nc.vector.tensor_copy(out=iota_f[:], in_=iota_i[:])
            iotap_f = big.tile([P, 1], f32, tag="iotapf")
            nc.vector.tensor_copy(out=iotap_f[:], in_=iotap_i[:])
            ident16 = big.tile([P, P], f16, tag="ident16")
            nc.vector.tensor_scalar(
                out=ident16[:], in0=iota_f[:], scalar1=iotap_f[:, 0:1],
                scalar2=None, op0=mybir.AluOpType.is_equal,
            )

            # xT: feature-major permuted own x (gather from x_full + transpose)
            xT = big.tile([P, ROWS], f16, tag="xT")
            for b in range(NB):
                xg = lp.tile([P, P], f16, tag="xg")
                nc.gpsimd.indirect_dma_start(
                    out=xg[:], out_offset=None, in_=x_full[:, :],
                    in_offset=bass.IndirectOffsetOnAxis(
                        ap=srci[:, C1 + b:C1 + b + 1], axis=0
                    ),
                )
                ptx = pp.tile([P, P], f16, tag="tr", name=f"ptx{b}")
                nc.tensor.transpose(out=ptx[:], in_=xg[:], identity=ident16[:])
                nc.vector.tensor_copy(out=xT[:, b * P:(b + 1) * P], in_=ptx[:])

            aggT_all = big.tile([P, ROWS], f16, tag="aggT")
            hT = [
                big.tile([P, ROWS], f16, tag=f"hT{j}", name=f"hT{j}")
                for j in range(2)
            ]
            z_sb = big.tile([P, 2 * NB], f32, tag="z")
            o_sb = big.tile([P, 2 * NB], f32, tag="o")
            out_sb = big.tile([P, 2 * NB], f32, tag="outs")
            z_own = dp.tile([ROWS, 2], f32, tag="zown")
            z_full = dp.tile([NP, 2], f32, tag="zfull")

            # ---- layer 1 aggregation: gather-accumulate per dst block ----
            col = 0
            for b in range(NB):
                w = nbk[b]
                agg = lp.tile([P, P], f16, tag="agg")
                for k in range(w):
                    nc.gpsimd.indirect_dma_start(
                        out=agg[:],
                        out_offset=None,
                        in_=x_full[:, :],
                        in_offset=bass.IndirectOffsetOnAxis(
                            ap=srci[:, col + k:col + k + 1], axis=0
                        ),
                        compute_op=(mybir.AluOpType.bypass if k == 0
                                    else mybir.AluOpType.add),
                    )
                aggm = lp.tile([P, P], f16, tag="aggm")
                nc.vector.tensor_scalar(
                    out=aggm[:], in0=agg[:], scalar1=recipf[:, b:b + 1],
                    scalar2=None, op0=mybir.AluOpType.mult,
                )
                ptr = pp.tile([P, P], f16, tag="tr")
                nc.tensor.transpose(out=ptr[:], in_=aggm[:], identity=ident16[:])
                nc.vector.tensor_copy(
                    out=aggT_all[:, b * P:(b + 1) * P], in_=ptr[:]
                )
                col += w

            # ---- layer 1 dense part, in 512-wide windows ----
            for wi in range(NW):
                lo = wi * WIN * P
                hi = min(ROWS, lo + WIN * P)
                for j in range(2):
                    ph = php.tile([P, hi - lo], f32, tag="h")
                    nc.tensor.matmul(
                        out=ph[:], lhsT=wp_sb[:, j * P:(j + 1) * P],
                        rhs=aggT_all[:, lo:hi], start=True, stop=False,
                    )
                    nc.tensor.matmul(
                        out=ph[:], lhsT=wp_sb[:, HID + j * P:HID + (j + 1) * P],
                        rhs=xT[:, lo:hi], start=False, stop=True,
                    )
                    nc.scalar.activation(
                        out=hT[j][:, lo:hi], in_=ph[:],
                        func=mybir.ActivationFunctionType.Relu,
                        bias=b1f[:, j:j + 1],
                    )

            # ---- z/o per block; z scattered to natural node order ----
            for b in range(NB):
                pzo = php.tile([P, 4], f32, tag="zo")
                for j in range(2):
                    nc.tensor.matmul(
                        out=pzo[:], lhsT=hT[j][:, b * P:(b + 1) * P],
                        rhs=wp_sb[:, 2 * HID + 4 * j:2 * HID + 4 * j + 4],
                        start=(j == 0), stop=(j == 1),
                    )
                nc.vector.tensor_copy(out=z_sb[:, 2 * b:2 * b + 2], in_=pzo[:, 0:2])
                nc.vector.tensor_tensor(
                    out=o_sb[:, 2 * b:2 * b + 2], in0=pzo[:, 2:4],
                    in1=b2f[:], op=mybir.AluOpType.add,
                )
                nc.gpsimd.indirect_dma_start(
                    out=z_own[:, :],
                    out_offset=bass.IndirectOffsetOnAxis(
                        ap=srci[:, C1 + NB + b:C1 + NB + b + 1], axis=0
                    ),
                    in_=z_sb[:, 2 * b:2 * b + 2],
                    in_offset=None,
                )

            nc.gpsimd.collective_compute(
                "AllGather",
                mybir.AluOpType.bypass,
                replica_groups=[list(range(NCORES))],
                ins=[z_own[:, :]],
                outs=[z_full[:, :]],
            )

            # ---- layer 2: gather-accumulate z, same table ----
            col = 0
            for b in range(NB):
                w = nbk[b]
                agg2 = lp.tile([P, 2], f32, tag="agg2")
                for k in range(w):
                    nc.gpsimd.indirect_dma_start(
                        out=agg2[:],
                        out_offset=None,
                        in_=z_full[:, :],
                        in_offset=bass.IndirectOffsetOnAxis(
                            ap=srci[:, col + k:col + k + 1], axis=0
                        ),
                        compute_op=(mybir.AluOpType.bypass if k == 0
                                    else mybir.AluOpType.add),
                    )
                red2 = lp.tile([P, 2], f32, tag="red2")
                nc.vector.tensor_scalar(
                    out=red2[:], in0=agg2[:], scalar1=recipf[:, b:b + 1],
                    scalar2=None, op0=mybir.AluOpType.mult,
                )
                nc.vector.tensor_tensor(
                    out=out_sb[:, 2 * b:2 * b + 2], in0=red2[:],
                    in1=o_sb[:, 2 * b:2 * b + 2], op=mybir.AluOpType.add,
                )
                col += w

            nc.sync.dma_start(out=out_d[:, :], in_=out_sb[:])
    nc.compile()
    return nc


def _run(inputs, repeat=1):
    in_maps, nbk, perm = _host_prep(**inputs)
    nc = _build(nbk)
    best = None
    res = None
    for _ in range(repeat):
        t0 = time.perf_counter()
        res = run_bass_kernel_spmd(
            nc, [dict(m) for m in in_maps], core_ids=list(range(NCORES))
        )
        dt = time.perf_counter() - t0
        best = dt if best is None else min(best, dt)
    full = np.empty((NP, 2), np.float32)
    for c in range(NCORES):
        a = res.results[c]["out"]  # [128, 98] in permuted order
        ap = a.reshape(P, NB, 2).transpose(1, 0, 2).reshape(ROWS, 2)
        full[c * ROWS + perm[c]] = ap
    return full[:N].astype(np.float32), best


def kernel(**inputs):
    # two runs: the first is cold (jit/executable load); the second is warm
    out, _ = _run(inputs, repeat=2)
    return out


# revision 19
# speedup vs baseline: 12.2181x; 1.2081x over previous
"""GraphSAGE 2-layer fraud detector on 8 trn2 NeuronCores.

Strategy (dst-partitioned, DMA scatter-accumulate; wire+instruction optimized):
  - Host->device wire dominates and is serial (~12ms/MB + per-tensor fixed
    cost), so inputs are minimal and few: one INT8 x shard per core
    (symmetric quantization, scale folded into the layer-1 recip table and
    into W1r on the host), one u16 edge/permutation table, one packed fp16
    weight tensor. x is AllGathered on-device; the gather-accumulate DMA
    convert-adds i8 rows into an fp16 accumulator (integer sums up to ~4K
    are near-exact in fp16). Final rel err ~1.2e-2 vs the 2e-2 gate.
  - Aggregation uses indirect-DMA gather with compute_op=add: each edge of
    dst-block b is assigned to (chunk k, partition = local dst position).
    Chunk 0 overwrites (bypass), later chunks accumulate, so
    agg[d, :] = sum_k x_full[srcq[d, k], :] with zero per-edge compute-engine
    work. Pad slots point at an all-zero x row.
  - Each core's dst nodes are SORTED BY IN-DEGREE before blocking, so the
    chunk count per block (= max in-block degree) tracks the block's degree
    quantile instead of the global max: ~820 chunks/layer instead of ~1470.
    All node-order-dependent state (recip, xT, z rows, out rows) follows the
    permutation; z is scattered back to natural node order via indirect DMA
    so layer 2 can gather by global node id, and the host inverse-permutes
    the final output.
  - Per block: scale agg by 1/deg, PE-transpose to feature-major; xT built
    by indirect-gathering the permuted rows from x_full + PE transpose.
    h = relu(W1l@aggT + W1r@xT + b1) in 512-wide windows; z = h@W2l.T,
    o = h@W2r.T + b2 per block. z AllGathered (50KB/core); layer 2 reuses
    the SAME srcq table to gather-accumulate the 2-wide z; out = agg2/deg+o.
"""

import os
import time

os.environ.setdefault("JAX_PLATFORMS", "cpu,axon")
os.environ.setdefault("NEURON_RT_RESET_CORES", "1")

import numpy as np

import concourse.bass as bass
import concourse.mybir as mybir
import concourse.tile as tile
from concourse import bacc
from concourse.bass_utils import run_bass_kernel_spmd

N = 50000
E = 800000
IN_C = 128
HID = 256
OUT_C = 2
NCORES = 8
P = 128
NB = 49                 # dst blocks per core
ROWS = NB * P           # 6272 rows per core
NP = NCORES * ROWS      # 50176 padded nodes
WIN = 4                 # dst blocks per h-matmul window
NW = (NB + WIN - 1) // WIN  # 13 windows (last is 1 block)

f32 = mybir.dt.float32
f16 = mybir.dt.float16
i32 = mybir.dt.int32
u16 = mybir.dt.uint16
i8 = mybir.dt.int8

WP = 2 * HID + 8 + 2 + 2 + 2 * NB  # W1lT | W1rT*s | Wzo | b1p | b2b | recip*s | recip


def _host_prep(x, edge_index, W1l, b1, W1r, W2l, b2, W2r):
    src = np.asarray(edge_index[0]).astype(np.int64)
    dst = np.asarray(edge_index[1]).astype(np.int64)
    cnt = np.bincount(dst, minlength=NP)
    recip = (1.0 / np.maximum(cnt, 1)).astype(np.float32)

    order = np.argsort(dst, kind="stable")
    s_src = src[order]
    starts = np.concatenate([[0], np.cumsum(cnt)])  # [NP+1]

    # per-core permutation: dsts sorted by in-degree (desc)
    cnt_c = cnt.reshape(NCORES, ROWS)
    perm = np.argsort(-cnt_c, axis=1, kind="stable")      # [c, pos] -> local dst
    pdeg = np.take_along_axis(cnt_c, perm, axis=1)        # degree at position
    nbk = np.maximum(pdeg.reshape(NCORES, NB, P).max(axis=2).max(axis=0), 1)
    C1 = int(nbk.sum())
    CT = C1 + 2 * NB     # + xT perm cols + z scatter cols

    srcq = np.full((NCORES, P, CT), N, dtype=np.uint16)
    for c in range(NCORES):
        col = 0
        for b in range(NB):
            w = int(nbk[b])
            for d in range(P):
                loc = int(perm[c, b * P + d])
                n0 = c * ROWS + loc
                k = int(cnt[n0])
                if k:
                    srcq[c, d, col:col + k] = s_src[starts[n0]:starts[n0] + k]
            col += w
        # xT gather cols: global x row of permuted position (b, d)
        srcq[c, :, C1:C1 + NB] = (c * ROWS + perm[c]).reshape(NB, P).T
        # z scatter cols: natural local row for permuted position (b, d)
        srcq[c, :, C1 + NB:] = perm[c].reshape(NB, P).T

    xf = np.asarray(x, dtype=np.float32)
    s_q = float(np.abs(xf).max()) / 127.0
    x_pad = np.zeros((NP, IN_C), np.int8)
    x_pad[:N] = np.clip(np.round(xf / s_q), -127, 127).astype(np.int8)

    wpack = np.zeros((P, WP), np.float16)
    wpack[:, 0:HID] = np.asarray(W1l).T.astype(np.float16)
    wpack[:, HID:2 * HID] = (np.asarray(W1r).T * s_q).astype(np.float16)
    for j in range(2):
        wpack[:, 2 * HID + 4 * j:2 * HID + 4 * j + 2] = \
            np.asarray(W2l).T[j * P:(j + 1) * P, :].astype(np.float16)
        wpack[:, 2 * HID + 4 * j + 2:2 * HID + 4 * j + 4] = \
            np.asarray(W2r).T[j * P:(j + 1) * P, :].astype(np.float16)
    wpack[:, 2 * HID + 8:2 * HID + 10] = \
        np.asarray(b1).reshape(2, P).T.astype(np.float16)
    wpack[:, 2 * HID + 10:2 * HID + 12] = \
        np.tile(np.asarray(b2).reshape(1, 2), (P, 1)).astype(np.float16)

    in_maps = []
    for c in range(NCORES):
        wpc = wpack.copy()
        rc = recip[c * ROWS:(c + 1) * ROWS][perm[c]]      # permuted recip
        rcb = rc.reshape(NB, P).T
        wpc[:, 2 * HID + 12:2 * HID + 12 + NB] = (rcb * s_q).astype(np.float16)
        wpc[:, 2 * HID + 12 + NB:] = rcb.astype(np.float16)
        m = {
            "srcq": np.ascontiguousarray(srcq[c]),
            "wpack": wpc,
            "xs": np.ascontiguousarray(x_pad[c * ROWS:(c + 1) * ROWS]),
        }
        in_maps.append(m)
    return in_maps, [int(v) for v in nbk], perm


def _build(nbk):
    C1 = sum(nbk)
    CT = C1 + 2 * NB
    nc = bacc.Bacc(None, target_bir_lowering=False, debug=False)

    xs_d = nc.dram_tensor("xs", [ROWS, IN_C], i8, kind="ExternalInput")
    srcq_d = nc.dram_tensor("srcq", [P, CT], u16, kind="ExternalInput")
    wpack_d = nc.dram_tensor("wpack", [P, WP], f16, kind="ExternalInput")
    out_d = nc.dram_tensor("out", [P, 2 * NB], f32, kind="ExternalOutput")

    with tile.TileContext(nc) as tc:
        with (
            tc.tile_pool(name="big", bufs=1) as big,
            tc.tile_pool(name="lp", bufs=4) as lp,
            tc.tile_pool(name="pp", bufs=2, space="PSUM") as pp,
            tc.tile_pool(name="php", bufs=2, space="PSUM") as php,
            tc.tile_pool(name="dram", bufs=1, space="DRAM") as dp,
        ):
            # ---- input staging ----
            srcu = big.tile([P, CT], u16, tag="srcu")
            nc.sync.dma_start(out=srcu[:], in_=srcq_d[:, :])
            wp_sb = big.tile([P, WP], f16, tag="wp")
            nc.sync.dma_start(out=wp_sb[:], in_=wpack_d[:, :])

            x_own = dp.tile([ROWS, IN_C], i8, tag="xown")
            nc.sync.dma_start(out=x_own[:, :], in_=xs_d[:, :])
            x_full = dp.tile([NP, IN_C], i8, tag="xfull")
            nc.gpsimd.collective_compute(
                "AllGather",
                mybir.AluOpType.bypass,
                replica_groups=[list(range(NCORES))],
                ins=[x_own[:, :]],
                outs=[x_full[:, :]],
            )

            srci = big.tile([P, CT], i32, tag="srci")
            nc.vector.tensor_copy(out=srci[:], in_=srcu[:])
            b1f = big.tile([P, 2], f32, tag="b1f")
            nc.vector.tensor_copy(out=b1f[:], in_=wp_sb[:, 2 * HID + 8:2 * HID + 10])
            b2f = big.tile([P, 2], f32, tag="b2f")
            nc.vector.tensor_copy(out=b2f[:], in_=wp_sb[:, 2 * HID + 10:2 * HID + 12])
            recipf = big.tile([P, NB], f32, tag="recipf")
            nc.vector.tensor_copy(
                out=recipf[:], in_=wp_sb[:, 2 * HID + 12:2 * HID + 12 + NB])
            recipf2 = big.tile([P, NB], f32, tag="recipf2")
            nc.vector.tensor_copy(
                out=recipf2[:], in_=wp_sb[:, 2 * HID + 12 + NB:])

            # identity (f16) for PE transposes
            iota_i = big.tile([P, P], i32, tag="iotai")
            nc.gpsimd.iota(out=iota_i[:], pattern=[[1, P]], base=0,
                           channel_multiplier=0)
            iotap_i = big.tile([P, 1], i32, tag="iotapi")
            nc.gpsimd.iota(out=iotap_i[:], pattern=[[0, 1]], base=0,
                           channel_multiplier=1)
            iota_f = big.tile([P, P], f32, tag="iotaf")
            nc.vector.tensor_copy(out=iota_f[:], in_=iota_i[:])
            iotap_f = big.tile([P, 1], f32, tag="iotapf")
            nc.vector.tensor_copy(out=iotap_f[:], in_=iotap_i[:])
            ident16 = big.tile([P, P], f16, tag="ident16")
            nc.vector.tensor_scalar(
                out=ident16[:], in0=iota_f[:], scalar1=iotap_f[:, 0:1],
                scalar2=None, op0=mybir.AluOpType.is_equal,
            )

            # xT: feature-major permuted own x (gather from x_full + transpose)
            xT = big.tile([P, ROWS], f16, tag="xT")
            for b in range(NB):
                xg = lp.tile([P, P], f16, tag="xg")
                nc.gpsimd.indirect_dma_start(
                    out=xg[:], out_offset=None, in_=x_full[:, :],
                    in_offset=bass.IndirectOffsetOnAxis(
                        ap=srci[:, C1 + b:C1 + b + 1], axis=0
                    ),
                )
                ptx = pp.tile([P, P], f16, tag="tr", name=f"ptx{b}")
                nc.tensor.transpose(out=ptx[:], in_=xg[:], identity=ident16[:])
                nc.vector.tensor_copy(out=xT[:, b * P:(b + 1) * P], in_=ptx[:])

            aggT_all = big.tile([P, ROWS], f16, tag="aggT")
            hT = [
                big.tile([P, ROWS], f16, tag=f"hT{j}", name=f"hT{j}")
                for j in range(2)
            ]
            z_sb = big.tile([P, 2 * NB], f32, tag="z")
            o_sb = big.tile([P, 2 * NB], f32, tag="o")
            out_sb = big.tile([P, 2 * NB], f32, tag="outs")
            z_own = dp.tile([ROWS, 2], f32, tag="zown")
            z_full = dp.tile([NP, 2], f32, tag="zfull")

            # ---- layer 1 aggregation: gather-accumulate per dst block ----
            col = 0
            for b in range(NB):
                w = nbk[b]
                agg = lp.tile([P, P], f16, tag="agg")
                for k in range(w):
                    nc.gpsimd.indirect_dma_start(
                        out=agg[:],
                        out_offset=None,
                        in_=x_full[:, :],
                        in_offset=bass.IndirectOffsetOnAxis(
                            ap=srci[:, col + k:col + k + 1], axis=0
                        ),
                        compute_op=(mybir.AluOpType.bypass if k == 0
                                    else mybir.AluOpType.add),
                    )
                aggm = lp.tile([P, P], f16, tag="aggm")
                nc.vector.tensor_scalar(
                    out=aggm[:], in0=agg[:], scalar1=recipf[:, b:b + 1],
                    scalar2=None, op0=mybir.AluOpType.mult,
                )
                ptr = pp.tile([P, P], f16, tag="tr")
                nc.tensor.transpose(out=ptr[:], in_=aggm[:], identity=ident16[:])
                nc.vector.tensor_copy(
                    out=aggT_all[:, b * P:(b + 1) * P], in_=ptr[:]
                )
                col += w

            # ---- layer 1 dense part, in 512-wide windows ----
            for wi in range(NW):
                lo = wi * WIN * P
                hi = min(ROWS, lo + WIN * P)
                for j in range(2):
                    ph = php.tile([P, hi - lo], f32, tag="h")
                    nc.tensor.matmul(
                        out=ph[:], lhsT=wp_sb[:, j * P:(j + 1) * P],
                        rhs=aggT_all[:, lo:hi], start=True, stop=False,
                    )
                    nc.tensor.matmul(
                        out=ph[:], lhsT=wp_sb[:, HID + j * P:HID + (j + 1) * P],
                        rhs=xT[:, lo:hi], start=False, stop=True,
                    )
                    nc.scalar.activation(
                        out=hT[j][:, lo:hi], in_=ph[:],
                        func=mybir.ActivationFunctionType.Relu,
                        bias=b1f[:, j:j + 1],
                    )

            # ---- z/o per block; z scattered to natural node order ----
            for b in range(NB):
                pzo = php.tile([P, 4], f32, tag="zo")
                for j in range(2):
                    nc.tensor.matmul(
                        out=pzo[:], lhsT=hT[j][:, b * P:(b + 1) * P],
                        rhs=wp_sb[:, 2 * HID + 4 * j:2 * HID + 4 * j + 4],
                        start=(j == 0), stop=(j == 1),
                    )
                nc.vector.tensor_copy(out=z_sb[:, 2 * b:2 * b + 2], in_=pzo[:, 0:2])
                nc.vector.tensor_tensor(
                    out=o_sb[:, 2 * b:2 * b + 2], in0=pzo[:, 2:4],
                    in1=b2f[:], op=mybir.AluOpType.add,
                )
                nc.gpsimd.indirect_dma_start(
                    out=z_own[:, :],
                    out_offset=bass.IndirectOffsetOnAxis(
                        ap=srci[:, C1 + NB + b:C1 + NB + b + 1], axis=0
                    ),
                    in_=z_sb[:, 2 * b:2 * b + 2],
                    in_offset=None,
                )

            nc.gpsimd.collective_compute(
                "AllGather",
                mybir.AluOpType.bypass,
                replica_groups=[list(range(NCORES))],
                ins=[z_own[:, :]],
                outs=[z_full[:, :]],
            )

            # ---- layer 2: gather-accumulate z, same table ----
            col = 0
            for b in range(NB):
                w = nbk[b]
                agg2 = lp.tile([P, 2], f32, tag="agg2")
                for k in range(w):
                    nc.gpsimd.indirect_dma_start(
                        out=agg2[:],
                        out_offset=None,
                        in_=z_full[:, :],
                        in_offset=bass.IndirectOffsetOnAxis(
                            ap=srci[:, col + k:col + k + 1], axis=0
                        ),
                        compute_op=(mybir.AluOpType.bypass if k == 0
                                    else mybir.AluOpType.add),
                    )
                red2 = lp.tile([P, 2], f32, tag="red2")
                nc.vector.tensor_scalar(
                    out=red2[:], in0=agg2[:], scalar1=recipf2[:, b:b + 1],
                    scalar2=None, op0=mybir.AluOpType.mult,
                )
                nc.vector.tensor_tensor(
                    out=out_sb[:, 2 * b:2 * b + 2], in0=red2[:],
                    in1=o_sb[:, 2 * b:2 * b + 2], op=mybir.AluOpType.add,
                )
                col += w

            nc.sync.dma_start(out=out_d[:, :], in_=out_sb[:])
    nc.compile()
    return nc


def _run(inputs, repeat=1):
    in_maps, nbk, perm = _host_prep(**inputs)
    nc = _build(nbk)
    best = None
    res = None
    for _ in range(repeat):
        t0 = time.perf_counter()
        res = run_bass_kernel_spmd(
            nc, [dict(m) for m in in_maps], core_ids=list(range(NCORES))
        )
        dt = time.perf_counter() - t0
        best = dt if best is None else min(best, dt)
    full = np.empty((NP, 2), np.float32)
    for c in range(NCORES):
        a = res.results[c]["out"]  # [128, 98] in permuted order
        ap = a.reshape(P, NB, 2).transpose(1, 0, 2).reshape(ROWS, 2)
        full[c * ROWS + perm[c]] = ap
    return full[:N].astype(np.float32), best


def kernel(**inputs):
    # multiple runs: the first is cold (jit/executable load); later are warm
    out, _ = _run(inputs, repeat=3)
    return out


# revision 20
# speedup vs baseline: 13.0414x; 1.0674x over previous
"""GraphSAGE 2-layer fraud detector on 8 trn2 NeuronCores.

Strategy (dst-partitioned, DMA scatter-accumulate; wire+instruction optimized):
  - Host->device wire dominates and is serial (~12ms/MB + per-tensor fixed
    cost), so inputs are minimal and few: one INT8 x shard per core
    (symmetric quantization, scale folded into the layer-1 recip table and
    into W1r on the host), one u16 edge/permutation table, one packed fp16
    weight tensor. x is AllGathered on-device; the gather-accumulate DMA
    convert-adds i8 rows into an fp16 accumulator (integer sums up to ~4K
    are near-exact in fp16). Final rel err ~1.2e-2 vs the 2e-2 gate.
  - Aggregation uses indirect-DMA gather with compute_op=add: each edge of
    dst-block b is assigned to (chunk k, partition = local dst position).
    Chunk 0 overwrites (bypass), later chunks accumulate, so
    agg[d, :] = sum_k x_full[srcq[d, k], :] with zero per-edge compute-engine
    work. Pad slots point at an all-zero x row.
  - Each core's dst nodes are SORTED BY IN-DEGREE before blocking, so the
    chunk count per block (= max in-block degree) tracks the block's degree
    quantile instead of the global max: ~820 chunks/layer instead of ~1470.
    All node-order-dependent state (recip, xT, z rows, out rows) follows the
    permutation; z is scattered back to natural node order via indirect DMA
    so layer 2 can gather by global node id, and the host inverse-permutes
    the final output.
  - Per block: scale agg by 1/deg, PE-transpose to feature-major; xT built
    by indirect-gathering the permuted rows from x_full + PE transpose.
    h = relu(W1l@aggT + W1r@xT + b1) in 512-wide windows; z = h@W2l.T,
    o = h@W2r.T + b2 per block. z AllGathered (50KB/core); layer 2 reuses
    the SAME srcq table to gather-accumulate the 2-wide z; out = agg2/deg+o.
"""

import os
import time

os.environ.setdefault("JAX_PLATFORMS", "cpu,axon")
os.environ.setdefault("NEURON_RT_RESET_CORES", "1")

import numpy as np

import concourse.bass as bass
import concourse.mybir as mybir
import concourse.tile as tile
from concourse import bacc
from concourse.bass_utils import run_bass_kernel_spmd

N = 50000
E = 800000
IN_C = 128
HID = 256
OUT_C = 2
NCORES = 8
P = 128
NB = 49                 # dst blocks per core
ROWS = NB * P           # 6272 rows per core
NP = NCORES * ROWS      # 50176 padded nodes
WIN = 4                 # dst blocks per h-matmul window
NW = (NB + WIN - 1) // WIN  # 13 windows (last is 1 block)

f32 = mybir.dt.float32
f16 = mybir.dt.float16
i32 = mybir.dt.int32
u16 = mybir.dt.uint16
i8 = mybir.dt.int8

WP = 2 * HID + 8 + 2 + 2 + 2 * NB  # W1lT | W1rT*s | Wzo | b1p | b2b | recip*s | recip


def _host_prep(x, edge_index, W1l, b1, W1r, W2l, b2, W2r):
    src = np.asarray(edge_index[0]).astype(np.int64)
    dst = np.asarray(edge_index[1]).astype(np.int64)
    cnt = np.bincount(dst, minlength=NP)
    recip = (1.0 / np.maximum(cnt, 1)).astype(np.float32)

    order = np.argsort(dst, kind="stable")
    s_src = src[order]
    starts = np.concatenate([[0], np.cumsum(cnt)])  # [NP+1]

    # per-core permutation: dsts sorted by in-degree (desc)
    cnt_c = cnt.reshape(NCORES, ROWS)
    perm = np.argsort(-cnt_c, axis=1, kind="stable")      # [c, pos] -> local dst
    pdeg = np.take_along_axis(cnt_c, perm, axis=1)        # degree at position
    nbk = np.maximum(pdeg.reshape(NCORES, NB, P).max(axis=2).max(axis=0), 1)
    C1 = int(nbk.sum())
    CT = C1 + 2 * NB     # + xT perm cols + z scatter cols

    srcq = np.full((NCORES, P, CT), N, dtype=np.uint16)
    for c in range(NCORES):
        col = 0
        for b in range(NB):
            w = int(nbk[b])
            for d in range(P):
                loc = int(perm[c, b * P + d])
                n0 = c * ROWS + loc
                k = int(cnt[n0])
                if k:
                    srcq[c, d, col:col + k] = s_src[starts[n0]:starts[n0] + k]
            col += w
        # xT gather cols: global x row of permuted position (b, d)
        srcq[c, :, C1:C1 + NB] = (c * ROWS + perm[c]).reshape(NB, P).T
        # z scatter cols: natural local row for permuted position (b, d)
        srcq[c, :, C1 + NB:] = perm[c].reshape(NB, P).T

    xf = np.asarray(x, dtype=np.float32)
    s_q = float(np.abs(xf).max()) / 127.0
    x_pad = np.zeros((NP, IN_C), np.int8)
    x_pad[:N] = np.clip(np.round(xf / s_q), -127, 127).astype(np.int8)

    wpack = np.zeros((P, WP), np.float16)
    wpack[:, 0:HID] = np.asarray(W1l).T.astype(np.float16)
    wpack[:, HID:2 * HID] = (np.asarray(W1r).T * s_q).astype(np.float16)
    for j in range(2):
        wpack[:, 2 * HID + 4 * j:2 * HID + 4 * j + 2] = \
            np.asarray(W2l).T[j * P:(j + 1) * P, :].astype(np.float16)
        wpack[:, 2 * HID + 4 * j + 2:2 * HID + 4 * j + 4] = \
            np.asarray(W2r).T[j * P:(j + 1) * P, :].astype(np.float16)
    wpack[:, 2 * HID + 8:2 * HID + 10] = \
        np.asarray(b1).reshape(2, P).T.astype(np.float16)
    wpack[:, 2 * HID + 10:2 * HID + 12] = \
        np.tile(np.asarray(b2).reshape(1, 2), (P, 1)).astype(np.float16)

    in_maps = []
    for c in range(NCORES):
        wpc = wpack.copy()
        rc = recip[c * ROWS:(c + 1) * ROWS][perm[c]]      # permuted recip
        rcb = rc.reshape(NB, P).T
        wpc[:, 2 * HID + 12:2 * HID + 12 + NB] = (rcb * s_q).astype(np.float16)
        wpc[:, 2 * HID + 12 + NB:] = rcb.astype(np.float16)
        m = {
            "srcq": np.ascontiguousarray(srcq[c]),
            "wpack": wpc,
            "xs": np.ascontiguousarray(x_pad[c * ROWS:(c + 1) * ROWS]),
        }
        in_maps.append(m)
    return in_maps, [int(v) for v in nbk], perm


def _build(nbk):
    C1 = sum(nbk)
    CT = C1 + 2 * NB
    nc = bacc.Bacc(None, target_bir_lowering=False, debug=False)

    xs_d = nc.dram_tensor("xs", [ROWS, IN_C], i8, kind="ExternalInput")
    srcq_d = nc.dram_tensor("srcq", [P, CT], u16, kind="ExternalInput")
    wpack_d = nc.dram_tensor("wpack", [P, WP], f16, kind="ExternalInput")
    out_d = nc.dram_tensor("out", [P, 2 * NB], f32, kind="ExternalOutput")

    with tile.TileContext(nc) as tc:
        with (
            tc.tile_pool(name="big", bufs=1) as big,
            tc.tile_pool(name="lp", bufs=4) as lp,
            tc.tile_pool(name="pp", bufs=2, space="PSUM") as pp,
            tc.tile_pool(name="php", bufs=2, space="PSUM") as php,
            tc.tile_pool(name="dram", bufs=1, space="DRAM") as dp,
        ):
            # ---- input staging ----
            srcu = big.tile([P, CT], u16, tag="srcu")
            nc.sync.dma_start(out=srcu[:], in_=srcq_d[:, :])
            wp_sb = big.tile([P, WP], f16, tag="wp")
            nc.sync.dma_start(out=wp_sb[:], in_=wpack_d[:, :])

            x_own = dp.tile([ROWS, IN_C], i8, tag="xown")
            nc.sync.dma_start(out=x_own[:, :], in_=xs_d[:, :])
            x_full = dp.tile([NP, IN_C], i8, tag="xfull")
            nc.gpsimd.collective_compute(
                "AllGather",
                mybir.AluOpType.bypass,
                replica_groups=[list(range(NCORES))],
                ins=[x_own[:, :]],
                outs=[x_full[:, :]],
            )

            srci = big.tile([P, CT], i32, tag="srci")
            nc.vector.tensor_copy(out=srci[:], in_=srcu[:])
            b1f = big.tile([P, 2], f32, tag="b1f")
            nc.vector.tensor_copy(out=b1f[:], in_=wp_sb[:, 2 * HID + 8:2 * HID + 10])
            b2f = big.tile([P, 2], f32, tag="b2f")
            nc.vector.tensor_copy(out=b2f[:], in_=wp_sb[:, 2 * HID + 10:2 * HID + 12])
            recipf = big.tile([P, NB], f32, tag="recipf")
            nc.vector.tensor_copy(
                out=recipf[:], in_=wp_sb[:, 2 * HID + 12:2 * HID + 12 + NB])
            recipf2 = big.tile([P, NB], f32, tag="recipf2")
            nc.vector.tensor_copy(
                out=recipf2[:], in_=wp_sb[:, 2 * HID + 12 + NB:])

            # identity (f16) for PE transposes
            iota_i = big.tile([P, P], i32, tag="iotai")
            nc.gpsimd.iota(out=iota_i[:], pattern=[[1, P]], base=0,
                           channel_multiplier=0)
            iotap_i = big.tile([P, 1], i32, tag="iotapi")
            nc.gpsimd.iota(out=iotap_i[:], pattern=[[0, 1]], base=0,
                           channel_multiplier=1)
            iota_f = big.tile([P, P], f32, tag="iotaf")
            nc.vector.tensor_copy(out=iota_f[:], in_=iota_i[:])
            iotap_f = big.tile([P, 1], f32, tag="iotapf")
            nc.vector.tensor_copy(out=iotap_f[:], in_=iotap_i[:])
            ident16 = big.tile([P, P], f16, tag="ident16")
            nc.vector.tensor_scalar(
                out=ident16[:], in0=iota_f[:], scalar1=iotap_f[:, 0:1],
                scalar2=None, op0=mybir.AluOpType.is_equal,
            )

            # xT: feature-major permuted own x (gather from x_full + transpose)
            xT = big.tile([P, ROWS], f16, tag="xT")
            for b in range(NB):
                xg = lp.tile([P, P], f16, tag="xg")
                nc.gpsimd.indirect_dma_start(
                    out=xg[:], out_offset=None, in_=x_full[:, :],
                    in_offset=bass.IndirectOffsetOnAxis(
                        ap=srci[:, C1 + b:C1 + b + 1], axis=0
                    ),
                )
                ptx = pp.tile([P, P], f16, tag="tr", name=f"ptx{b}")
                nc.tensor.transpose(out=ptx[:], in_=xg[:], identity=ident16[:])
                nc.vector.tensor_copy(out=xT[:, b * P:(b + 1) * P], in_=ptx[:])

            aggT_all = big.tile([P, ROWS], f16, tag="aggT")
            hT = [
                big.tile([P, ROWS], f16, tag=f"hT{j}", name=f"hT{j}")
                for j in range(2)
            ]
            z_sb = big.tile([P, 2 * NB], f16, tag="z")
            o_sb = big.tile([P, 2 * NB], f32, tag="o")
            out_sb = big.tile([P, 2 * NB], f32, tag="outs")
            z_own = dp.tile([ROWS, 2], f16, tag="zown")
            z_full = dp.tile([NP, 2], f16, tag="zfull")

            # ---- layer 1 aggregation: gather-accumulate per dst block ----
            col = 0
            for b in range(NB):
                w = nbk[b]
                agg = lp.tile([P, P], f16, tag="agg")
                for k in range(w):
                    nc.gpsimd.indirect_dma_start(
                        out=agg[:],
                        out_offset=None,
                        in_=x_full[:, :],
                        in_offset=bass.IndirectOffsetOnAxis(
                            ap=srci[:, col + k:col + k + 1], axis=0
                        ),
                        compute_op=(mybir.AluOpType.bypass if k == 0
                                    else mybir.AluOpType.add),
                    )
                aggm = lp.tile([P, P], f16, tag="aggm")
                nc.vector.tensor_scalar(
                    out=aggm[:], in0=agg[:], scalar1=recipf[:, b:b + 1],
                    scalar2=None, op0=mybir.AluOpType.mult,
                )
                ptr = pp.tile([P, P], f16, tag="tr")
                nc.tensor.transpose(out=ptr[:], in_=aggm[:], identity=ident16[:])
                nc.vector.tensor_copy(
                    out=aggT_all[:, b * P:(b + 1) * P], in_=ptr[:]
                )
                col += w

            # ---- layer 1 dense part, in 512-wide windows ----
            for wi in range(NW):
                lo = wi * WIN * P
                hi = min(ROWS, lo + WIN * P)
                for j in range(2):
                    ph = php.tile([P, hi - lo], f32, tag="h")
                    nc.tensor.matmul(
                        out=ph[:], lhsT=wp_sb[:, j * P:(j + 1) * P],
                        rhs=aggT_all[:, lo:hi], start=True, stop=False,
                    )
                    nc.tensor.matmul(
                        out=ph[:], lhsT=wp_sb[:, HID + j * P:HID + (j + 1) * P],
                        rhs=xT[:, lo:hi], start=False, stop=True,
                    )
                    nc.scalar.activation(
                        out=hT[j][:, lo:hi], in_=ph[:],
                        func=mybir.ActivationFunctionType.Relu,
                        bias=b1f[:, j:j + 1],
                    )

            # ---- z/o per block; z scattered to natural node order ----
            for b in range(NB):
                pzo = php.tile([P, 4], f32, tag="zo")
                for j in range(2):
                    nc.tensor.matmul(
                        out=pzo[:], lhsT=hT[j][:, b * P:(b + 1) * P],
                        rhs=wp_sb[:, 2 * HID + 4 * j:2 * HID + 4 * j + 4],
                        start=(j == 0), stop=(j == 1),
                    )
                nc.vector.tensor_copy(out=z_sb[:, 2 * b:2 * b + 2], in_=pzo[:, 0:2])
                nc.vector.tensor_tensor(
                    out=o_sb[:, 2 * b:2 * b + 2], in0=pzo[:, 2:4],
                    in1=b2f[:], op=mybir.AluOpType.add,
                )
                nc.gpsimd.indirect_dma_start(
                    out=z_own[:, :],
                    out_offset=bass.IndirectOffsetOnAxis(
                        ap=srci[:, C1 + NB + b:C1 + NB + b + 1], axis=0
                    ),
                    in_=z_sb[:, 2 * b:2 * b + 2],
                    in_offset=None,
                )

            nc.gpsimd.collective_compute(
                "AllGather",
                mybir.AluOpType.bypass,
                replica_groups=[list(range(NCORES))],
                ins=[z_own[:, :]],
                outs=[z_full[:, :]],
            )

            # ---- layer 2: gather-accumulate z, same table ----
            col = 0
            for b in range(NB):
                w = nbk[b]
                agg2 = lp.tile([P, 2], f32, tag="agg2")
                for k in range(w):
                    nc.gpsimd.indirect_dma_start(
                        out=agg2[:],
                        out_offset=None,
                        in_=z_full[:, :],
                        in_offset=bass.IndirectOffsetOnAxis(
                            ap=srci[:, col + k:col + k + 1], axis=0
                        ),
                        compute_op=(mybir.AluOpType.bypass if k == 0
                                    else mybir.AluOpType.add),
                    )
                red2 = lp.tile([P, 2], f32, tag="red2")
                nc.vector.tensor_scalar(
                    out=red2[:], in0=agg2[:], scalar1=recipf2[:, b:b + 1],
                    scalar2=None, op0=mybir.AluOpType.mult,
                )
                nc.vector.tensor_tensor(
                    out=out_sb[:, 2 * b:2 * b + 2], in0=red2[:],
                    in1=o_sb[:, 2 * b:2 * b + 2], op=mybir.AluOpType.add,
                )
                col += w

            nc.sync.dma_start(out=out_d[:, :], in_=out_sb[:])
    nc.compile()
    return nc


def _run(inputs, repeat=1):
    in_maps, nbk, perm = _host_prep(**inputs)
    nc = _build(nbk)
    best = None
    res = None
    for _ in range(repeat):
        t0 = time.perf_counter()
        res = run_bass_kernel_spmd(
            nc, [dict(m) for m in in_maps], core_ids=list(range(NCORES))
        )
        dt = time.perf_counter() - t0
        best = dt if best is None else min(best, dt)
    full = np.empty((NP, 2), np.float32)
    for c in range(NCORES):
        a = res.results[c]["out"]  # [128, 98] in permuted order
        ap = a.reshape(P, NB, 2).transpose(1, 0, 2).reshape(ROWS, 2)
        full[c * ROWS + perm[c]] = ap
    return full[:N].astype(np.float32), best


def kernel(**inputs):
    # multiple runs: the first is cold (jit/executable load); later are warm
    out, _ = _run(inputs, repeat=3)
    return out


# revision 21
# speedup vs baseline: 13.6960x; 1.0502x over previous
"""GraphSAGE 2-layer fraud detector on 8 trn2 NeuronCores.

Strategy (dst-partitioned, DMA scatter-accumulate; wire+instruction optimized):
  - Host->device wire dominates and is serial (~12ms/MB + per-tensor fixed
    cost), so inputs are minimal and few: one INT8 x shard per core
    (symmetric quantization, scale folded into the layer-1 recip table and
    into W1r on the host), one u16 edge/permutation table, one packed fp16
    weight tensor. x is AllGathered on-device; the gather-accumulate DMA
    convert-adds i8 rows into an fp16 accumulator (integer sums up to ~4K
    are near-exact in fp16). Final rel err ~1.2e-2 vs the 2e-2 gate.
  - Aggregation uses indirect-DMA gather with compute_op=add: each edge of
    dst-block b is assigned to (chunk k, partition = local dst position).
    Chunk 0 overwrites (bypass), later chunks accumulate, so
    agg[d, :] = sum_k x_full[srcq[d, k], :] with zero per-edge compute-engine
    work. Pad slots point at an all-zero x row.
  - Each core's dst nodes are SORTED BY IN-DEGREE before blocking, so the
    chunk count per block (= max in-block degree) tracks the block's degree
    quantile instead of the global max: ~820 chunks/layer instead of ~1470.
    All node-order-dependent state (recip, xT, z rows, out rows) follows the
    permutation; z is scattered back to natural node order via indirect DMA
    so layer 2 can gather by global node id, and the host inverse-permutes
    the final output.
  - Per block: scale agg by 1/deg, PE-transpose to feature-major; xT built
    by indirect-gathering the permuted rows from x_full + PE transpose.
    h = relu(W1l@aggT + W1r@xT + b1) in 512-wide windows; z = h@W2l.T,
    o = h@W2r.T + b2 per block. z AllGathered (50KB/core); layer 2 reuses
    the SAME srcq table to gather-accumulate the 2-wide z; out = agg2/deg+o.
"""

import os
import time

os.environ.setdefault("JAX_PLATFORMS", "cpu,axon")
os.environ.setdefault("NEURON_RT_RESET_CORES", "1")

import numpy as np

import concourse.bass as bass
import concourse.mybir as mybir
import concourse.tile as tile
from concourse import bacc
from concourse.bass_utils import run_bass_kernel_spmd

N = 50000
E = 800000
IN_C = 128
HID = 256
OUT_C = 2
NCORES = 8
P = 128
NB = 49                 # dst blocks per core
ROWS = NB * P           # 6272 rows per core
NP = NCORES * ROWS      # 50176 padded nodes
WIN = 4                 # dst blocks per h-matmul window
NW = (NB + WIN - 1) // WIN  # 13 windows (last is 1 block)

f32 = mybir.dt.float32
f16 = mybir.dt.float16
i32 = mybir.dt.int32
u16 = mybir.dt.uint16
i8 = mybir.dt.int8

WP = 2 * HID + 8 + 2 + 2 + 2 * NB  # W1lT | W1rT*s | Wzo | b1p | b2b | recip*s | recip


def _host_prep(x, edge_index, W1l, b1, W1r, W2l, b2, W2r):
    src = np.asarray(edge_index[0]).astype(np.int64)
    dst = np.asarray(edge_index[1]).astype(np.int64)
    cnt = np.bincount(dst, minlength=NP)
    recip = (1.0 / np.maximum(cnt, 1)).astype(np.float32)

    order = np.argsort(dst, kind="stable")
    s_src = src[order]
    starts = np.concatenate([[0], np.cumsum(cnt)])  # [NP+1]

    # per-core permutation: dsts sorted by in-degree (desc)
    cnt_c = cnt.reshape(NCORES, ROWS)
    perm = np.argsort(-cnt_c, axis=1, kind="stable")      # [c, pos] -> local dst
    pdeg = np.take_along_axis(cnt_c, perm, axis=1)        # degree at position
    nbk = np.maximum(pdeg.reshape(NCORES, NB, P).max(axis=2).max(axis=0), 1)
    C1 = int(nbk.sum())
    CT = C1 + 2 * NB     # + xT perm cols + z scatter cols

    srcq = np.full((NCORES, P, CT), N, dtype=np.uint16)
    for c in range(NCORES):
        col = 0
        for b in range(NB):
            w = int(nbk[b])
            for d in range(P):
                loc = int(perm[c, b * P + d])
                n0 = c * ROWS + loc
                k = int(cnt[n0])
                if k:
                    srcq[c, d, col:col + k] = s_src[starts[n0]:starts[n0] + k]
            col += w
        # xT gather cols: global x row of permuted position (b, d)
        srcq[c, :, C1:C1 + NB] = (c * ROWS + perm[c]).reshape(NB, P).T
        # z scatter cols: natural local row for permuted position (b, d)
        srcq[c, :, C1 + NB:] = perm[c].reshape(NB, P).T

    xf = np.asarray(x, dtype=np.float32)
    s_q = float(np.abs(xf).max()) / 127.0
    x_pad = np.zeros((NP, IN_C), np.int8)
    x_pad[:N] = np.clip(np.round(xf / s_q), -127, 127).astype(np.int8)

    wpack = np.zeros((P, WP), np.float16)
    wpack[:, 0:HID] = np.asarray(W1l).T.astype(np.float16)
    wpack[:, HID:2 * HID] = (np.asarray(W1r).T * s_q).astype(np.float16)
    for j in range(2):
        wpack[:, 2 * HID + 4 * j:2 * HID + 4 * j + 2] = \
            np.asarray(W2l).T[j * P:(j + 1) * P, :].astype(np.float16)
        wpack[:, 2 * HID + 4 * j + 2:2 * HID + 4 * j + 4] = \
            np.asarray(W2r).T[j * P:(j + 1) * P, :].astype(np.float16)
    wpack[:, 2 * HID + 8:2 * HID + 10] = \
        np.asarray(b1).reshape(2, P).T.astype(np.float16)
    wpack[:, 2 * HID + 10:2 * HID + 12] = \
        np.tile(np.asarray(b2).reshape(1, 2), (P, 1)).astype(np.float16)

    in_maps = []
    for c in range(NCORES):
        wpc = wpack.copy()
        rc = recip[c * ROWS:(c + 1) * ROWS][perm[c]]      # permuted recip
        rcb = rc.reshape(NB, P).T
        wpc[:, 2 * HID + 12:2 * HID + 12 + NB] = (rcb * s_q).astype(np.float16)
        wpc[:, 2 * HID + 12 + NB:] = rcb.astype(np.float16)
        m = {
            "srcq": np.ascontiguousarray(srcq[c]),
            "wpack": wpc,
            "xs": np.ascontiguousarray(x_pad[c * ROWS:(c + 1) * ROWS]),
        }
        in_maps.append(m)
    return in_maps, [int(v) for v in nbk], perm


def _build(nbk):
    C1 = sum(nbk)
    CT = C1 + 2 * NB
    nc = bacc.Bacc(None, target_bir_lowering=False, debug=False)

    xs_d = nc.dram_tensor("xs", [ROWS, IN_C], i8, kind="ExternalInput")
    srcq_d = nc.dram_tensor("srcq", [P, CT], u16, kind="ExternalInput")
    wpack_d = nc.dram_tensor("wpack", [P, WP], f16, kind="ExternalInput")
    out_d = nc.dram_tensor("out", [P, 2 * NB], f32, kind="ExternalOutput")

    with tile.TileContext(nc) as tc:
        with (
            tc.tile_pool(name="big", bufs=1) as big,
            tc.tile_pool(name="lp", bufs=4) as lp,
            tc.tile_pool(name="pp", bufs=2, space="PSUM") as pp,
            tc.tile_pool(name="php", bufs=2, space="PSUM") as php,
            tc.tile_pool(name="dram", bufs=1, space="DRAM") as dp,
        ):
            # ---- input staging ----
            srcu = big.tile([P, CT], u16, tag="srcu")
            nc.sync.dma_start(out=srcu[:], in_=srcq_d[:, :])
            wp_sb = big.tile([P, WP], f16, tag="wp")
            nc.sync.dma_start(out=wp_sb[:], in_=wpack_d[:, :])

            x_own = dp.tile([ROWS, IN_C], i8, tag="xown")
            nc.sync.dma_start(out=x_own[:, :], in_=xs_d[:, :])
            x_full = dp.tile([NP, IN_C], i8, tag="xfull")
            nc.gpsimd.collective_compute(
                "AllGather",
                mybir.AluOpType.bypass,
                replica_groups=[list(range(NCORES))],
                ins=[x_own[:, :]],
                outs=[x_full[:, :]],
            )

            srci = big.tile([P, CT], i32, tag="srci")
            nc.vector.tensor_copy(out=srci[:], in_=srcu[:])
            b1f = big.tile([P, 2], f32, tag="b1f")
            nc.vector.tensor_copy(out=b1f[:], in_=wp_sb[:, 2 * HID + 8:2 * HID + 10])
            b2f = big.tile([P, 2], f32, tag="b2f")
            nc.vector.tensor_copy(out=b2f[:], in_=wp_sb[:, 2 * HID + 10:2 * HID + 12])
            recipf = big.tile([P, NB], f32, tag="recipf")
            nc.vector.tensor_copy(
                out=recipf[:], in_=wp_sb[:, 2 * HID + 12:2 * HID + 12 + NB])
            recipf2 = big.tile([P, NB], f32, tag="recipf2")
            nc.vector.tensor_copy(
                out=recipf2[:], in_=wp_sb[:, 2 * HID + 12 + NB:])

            # identity (f16) for PE transposes
            iota_i = big.tile([P, P], i32, tag="iotai")
            nc.gpsimd.iota(out=iota_i[:], pattern=[[1, P]], base=0,
                           channel_multiplier=0)
            iotap_i = big.tile([P, 1], i32, tag="iotapi")
            nc.gpsimd.iota(out=iotap_i[:], pattern=[[0, 1]], base=0,
                           channel_multiplier=1)
            iota_f = big.tile([P, P], f32, tag="iotaf")
            nc.vector.tensor_copy(out=iota_f[:], in_=iota_i[:])
            iotap_f = big.tile([P, 1], f32, tag="iotapf")
            nc.vector.tensor_copy(out=iotap_f[:], in_=iotap_i[:])
            ident16 = big.tile([P, P], f16, tag="ident16")
            nc.vector.tensor_scalar(
                out=ident16[:], in0=iota_f[:], scalar1=iotap_f[:, 0:1],
                scalar2=None, op0=mybir.AluOpType.is_equal,
            )

            # xT: feature-major permuted own x (gather from x_full + transpose)
            xT = big.tile([P, ROWS], f16, tag="xT")
            for b in range(NB):
                xg = lp.tile([P, P], f16, tag="xg")
                nc.gpsimd.indirect_dma_start(
                    out=xg[:], out_offset=None, in_=x_full[:, :],
                    in_offset=bass.IndirectOffsetOnAxis(
                        ap=srci[:, C1 + b:C1 + b + 1], axis=0
                    ),
                )
                ptx = pp.tile([P, P], f16, tag="tr", name=f"ptx{b}")
                nc.tensor.transpose(out=ptx[:], in_=xg[:], identity=ident16[:])
                nc.vector.tensor_copy(out=xT[:, b * P:(b + 1) * P], in_=ptx[:])

            aggT_all = big.tile([P, ROWS], f16, tag="aggT")
            hT = [
                big.tile([P, ROWS], f16, tag=f"hT{j}", name=f"hT{j}")
                for j in range(2)
            ]
            z_sb = big.tile([P, 2 * NB], f16, tag="z")
            o_sb = big.tile([P, 2 * NB], f32, tag="o")
            out_sb = big.tile([P, 2 * NB], f32, tag="outs")
            z_own = dp.tile([ROWS, 2], f16, tag="zown")
            z_full = dp.tile([NP, 2], f16, tag="zfull")

            # ---- layer 1 aggregation: gather-accumulate per dst block ----
            agg_all = big.tile([P, ROWS], f16, tag="aggall")
            col = 0
            for b in range(NB):
                w = nbk[b]
                for k in range(w):
                    nc.gpsimd.indirect_dma_start(
                        out=agg_all[:, b * P:(b + 1) * P],
                        out_offset=None,
                        in_=x_full[:, :],
                        in_offset=bass.IndirectOffsetOnAxis(
                            ap=srci[:, col + k:col + k + 1], axis=0
                        ),
                        compute_op=(mybir.AluOpType.bypass if k == 0
                                    else mybir.AluOpType.add),
                    )
                col += w
            nc.vector.tensor_tensor(
                out=agg_all[:, :].rearrange("p (b f) -> p b f", f=P),
                in0=agg_all[:, :].rearrange("p (b f) -> p b f", f=P),
                in1=recipf[:, :].to_broadcast([P, NB, P]),
                op=mybir.AluOpType.mult,
            )
            for b in range(NB):
                ptr = pp.tile([P, P], f16, tag="tr")
                nc.tensor.transpose(
                    out=ptr[:], in_=agg_all[:, b * P:(b + 1) * P],
                    identity=ident16[:],
                )
                nc.vector.tensor_copy(
                    out=aggT_all[:, b * P:(b + 1) * P], in_=ptr[:]
                )

            # ---- layer 1 dense part, in 512-wide windows ----
            for wi in range(NW):
                lo = wi * WIN * P
                hi = min(ROWS, lo + WIN * P)
                for j in range(2):
                    ph = php.tile([P, hi - lo], f32, tag="h")
                    nc.tensor.matmul(
                        out=ph[:], lhsT=wp_sb[:, j * P:(j + 1) * P],
                        rhs=aggT_all[:, lo:hi], start=True, stop=False,
                    )
                    nc.tensor.matmul(
                        out=ph[:], lhsT=wp_sb[:, HID + j * P:HID + (j + 1) * P],
                        rhs=xT[:, lo:hi], start=False, stop=True,
                    )
                    nc.scalar.activation(
                        out=hT[j][:, lo:hi], in_=ph[:],
                        func=mybir.ActivationFunctionType.Relu,
                        bias=b1f[:, j:j + 1],
                    )

            # ---- z/o per block; z scattered to natural node order ----
            for b in range(NB):
                pzo = php.tile([P, 4], f32, tag="zo")
                for j in range(2):
                    nc.tensor.matmul(
                        out=pzo[:], lhsT=hT[j][:, b * P:(b + 1) * P],
                        rhs=wp_sb[:, 2 * HID + 4 * j:2 * HID + 4 * j + 4],
                        start=(j == 0), stop=(j == 1),
                    )
                nc.vector.tensor_copy(out=z_sb[:, 2 * b:2 * b + 2], in_=pzo[:, 0:2])
                nc.vector.tensor_tensor(
                    out=o_sb[:, 2 * b:2 * b + 2], in0=pzo[:, 2:4],
                    in1=b2f[:], op=mybir.AluOpType.add,
                )
                nc.gpsimd.indirect_dma_start(
                    out=z_own[:, :],
                    out_offset=bass.IndirectOffsetOnAxis(
                        ap=srci[:, C1 + NB + b:C1 + NB + b + 1], axis=0
                    ),
                    in_=z_sb[:, 2 * b:2 * b + 2],
                    in_offset=None,
                )

            nc.gpsimd.collective_compute(
                "AllGather",
                mybir.AluOpType.bypass,
                replica_groups=[list(range(NCORES))],
                ins=[z_own[:, :]],
                outs=[z_full[:, :]],
            )

            # ---- layer 2: gather-accumulate z, same table ----
            agg2_all = big.tile([P, 2 * NB], f32, tag="agg2all")
            col = 0
            for b in range(NB):
                w = nbk[b]
                for k in range(w):
                    nc.gpsimd.indirect_dma_start(
                        out=agg2_all[:, 2 * b:2 * b + 2],
                        out_offset=None,
                        in_=z_full[:, :],
                        in_offset=bass.IndirectOffsetOnAxis(
                            ap=srci[:, col + k:col + k + 1], axis=0
                        ),
                        compute_op=(mybir.AluOpType.bypass if k == 0
                                    else mybir.AluOpType.add),
                    )
                col += w
            nc.vector.tensor_tensor(
                out=agg2_all[:, :].rearrange("p (b j) -> p b j", j=2),
                in0=agg2_all[:, :].rearrange("p (b j) -> p b j", j=2),
                in1=recipf2[:, :].to_broadcast([P, NB, 2]),
                op=mybir.AluOpType.mult,
            )
            nc.vector.tensor_tensor(
                out=out_sb[:], in0=agg2_all[:], in1=o_sb[:],
                op=mybir.AluOpType.add,
            )

            nc.sync.dma_start(out=out_d[:, :], in_=out_sb[:])
    nc.compile()
    return nc


def _run(inputs, repeat=1):
    in_maps, nbk, perm = _host_prep(**inputs)
    nc = _build(nbk)
    best = None
    res = None
    for _ in range(repeat):
        t0 = time.perf_counter()
        res = run_bass_kernel_spmd(
            nc, [dict(m) for m in in_maps], core_ids=list(range(NCORES))
        )
        dt = time.perf_counter() - t0
        best = dt if best is None else min(best, dt)
    full = np.empty((NP, 2), np.float32)
    for c in range(NCORES):
        a = res.results[c]["out"]  # [128, 98] in permuted order
        ap = a.reshape(P, NB, 2).transpose(1, 0, 2).reshape(ROWS, 2)
        full[c * ROWS + perm[c]] = ap
    return full[:N].astype(np.float32), best


def kernel(**inputs):
    # multiple runs: the first is cold (jit/executable load); later are warm
    out, _ = _run(inputs, repeat=3)
    return out
